# revision 1
# baseline (speedup 1.0000x reference)
"""CGMM (Contextual Graph Markov Model) forward pass on 8 Trainium2 NeuronCores.

Self-contained: takes FULL inputs as numpy arrays, shards nodes/edges across
the 8 cores (graph parallel), runs a Bass/Tile kernel via
run_bass_kernel_spmd, returns the FULL [N, L, G] log-likelihood output.

Algorithm layout (per core, nodes on partitions, cg = g*8 + c on free dim):
  layer 0:  u0[n, cg] = B0[c, x_n, g]*Pi[c, g]  via one-hot(x) matmul
            Z = sum_c u, ll0 = log Z, h = u/Z  (h stored bf16, row-major)
  layers 1..3:
            all-gather h across cores  ->  h_full [N, 128] bf16 (Shared DRAM)
            gather h_full[src] per edge (dma_gather, 256B rows)
            aggr[dst, cg] = segment-sum via one-hot(dst_local) matmuls (PSUM fp32)
            cnt from row-sums of aggr (h rows sum to G exactly)
            QA = Qbig @ aggr^T (PE transpose + fp32 matmul)
            u = Bx * QA; Z = sum_c u; ll = log Z - log(cnt); h = u/Z
Edge streams are host-preprocessed: sorted by (dst block, src half), padded to
a cross-core-uniform tile schedule; padded slots gather row 0 with
dst_local = -1 (one-hot row of zeros -> no contribution).
"""
import os
import sys

sys.path.insert(0, "/opt/trn_rl_repo")

import numpy as np
import ml_dtypes

BF = ml_dtypes.bfloat16

# ---- problem sizes (hardcoded per contract) --------------------------------
N, E, C, M, G, L = 50000, 800000, 8, 32, 16, 4
NCORES = 8
CG = C * G  # 128


class Cfg:
    def __init__(self, n=N, e=E, ncores=NCORES, tg=32):
        self.n = n
        self.e = e
        self.ncores = ncores
        self.npc = n // ncores
        self.nb = (self.npc + 127) // 128
        self.half = n // 2
        self.tg = tg  # gather chunk size in 128-edge tiles
        self.lo_nb = (self.nb + 1) // 2  # blocks in the lo bank


# ---- host preprocessing -----------------------------------------------------

def preprocess(x, edge_index, cfg):
    """Build per-core aux arrays + the (cross-core uniform) tile schedule."""
    dst = np.asarray(edge_index[0], dtype=np.int64)
    src = np.asarray(edge_index[1], dtype=np.int64)
    x = np.asarray(x, dtype=np.int64)
    nc_, npc, nb, half = cfg.ncores, cfg.npc, cfg.nb, cfg.half

    lo_nb = cfg.lo_nb
    LO = lo_nb * 128
    HI = npc - LO
    owner = dst // npc
    per_core = []
    cntAB = np.zeros((nc_, nb, 2), dtype=np.int64)
    for c in range(nc_):
        sel = owner == c
        d = dst[sel] - c * npc
        s = src[sel]
        b = d // 128
        order = np.argsort(b, kind="stable")
        b, d, s = b[order], d[order], s[order]
        dl = d % 128
        sown = s // npc
        soff = s % npc
        hf = (soff >= LO).astype(np.int64)
        # bank row ids
        s = np.where(hf == 0, sown * LO + soff, sown * HI + (soff - LO))
        per_core.append((b, dl, s, hf))
        # counts per (block, half)
        key = b * 2 + hf
        cnt = np.bincount(key, minlength=nb * 2).reshape(nb, 2)
        cntAB[c] = cnt
    TA = np.maximum(1, -(-cntAB[:, :, 0].max(axis=0) // 128))
    TB = np.maximum(1, -(-cntAB[:, :, 1].max(axis=0) // 128))
    totTA, totTB = int(TA.sum()), int(TB.sum())
    offA = np.concatenate([[0], np.cumsum(TA)]).astype(np.int64)  # tile offsets
    offB = np.concatenate([[0], np.cumsum(TB)]).astype(np.int64)

    cores = []
    for c in range(nc_):
        b, dl, s, hf = per_core[c]
        idxA = np.zeros(totTA * 128, dtype=np.int64)
        dlA = np.full(totTA * 128, -1, dtype=np.int64)
        idxB = np.zeros(totTB * 128, dtype=np.int64)
        dlB = np.full(totTB * 128, -1, dtype=np.int64)
        for bb in range(nb):
            mA = (b == bb) & (hf == 0)
            mB = (b == bb) & (hf == 1)
            nA, nB_ = int(mA.sum()), int(mB.sum())
            a0, b0 = offA[bb] * 128, offB[bb] * 128
            idxA[a0:a0 + nA] = s[mA]
            dlA[a0:a0 + nA] = dl[mA]
            idxB[b0:b0 + nB_] = s[mB]
            dlB[b0:b0 + nB_] = dl[mB]

        # idx dram layout: [128, cols] int16; index i at [i%16, i//16], the
        # 16-row block replicated 8x down the partitions (one copy per Q7 core)
        allidx = np.concatenate([idxA, idxB]).astype(np.int16)
        idx16 = allidx.reshape(-1, 16).T  # [16, tot/16]
        idx_d = np.tile(idx16, (8, 1))    # [128, tot/16]

        # dstloc dram layout: [128, T_tot] bf16, partition = slot within tile
        alldl = np.concatenate([dlA, dlB]).astype(np.float32)
        dl_d = alldl.reshape(-1, 128).T.copy()  # [128, T_tot] fp32

        # x dram layout: [128, nb], partition-major, fp32
        xloc = np.zeros(nb * 128, dtype=np.float32)
        xloc[:npc] = x[c * npc:(c + 1) * npc]
        x_d = xloc.reshape(nb, 128).T.copy()  # [128, nb]

        cores.append({"idx": np.ascontiguousarray(idx_d),
                      "dstloc": np.ascontiguousarray(dl_d),
                      "xq": np.ascontiguousarray(x_d)})
    return cores, TA.astype(int), TB.astype(int)


def permute_params(lambda_B0, lambda_Pi, lambda_Q, lambda_B):
    """Pure layout permutations (no compute): partition (g, c/k)-major views."""
    lamB0p = np.ascontiguousarray(
        np.transpose(np.asarray(lambda_B0, np.float32), (2, 0, 1)).reshape(G * C, M))
    lamPip = np.ascontiguousarray(np.asarray(lambda_Pi, np.float32).T)  # [G, C]
    lamQp = np.ascontiguousarray(
        np.transpose(np.asarray(lambda_Q, np.float32), (0, 3, 2, 1)).reshape(
            L - 1, G * C, C))
    lamBp = np.ascontiguousarray(
        np.transpose(np.asarray(lambda_B, np.float32), (0, 3, 1, 2)).reshape(
            L - 1, G * C, M))
    return {"lamB0p": lamB0p, "lamPip": lamPip, "lamQp": lamQp, "lamBp": lamBp}


def make_consts():
    iota_f = np.tile(np.arange(128, dtype=np.float32), (128, 1))
    iota_b = iota_f.astype(BF)
    ident_f = np.eye(128, dtype=np.float32)
    # maskg[p, f] = 1 if p//8 == f//8 (same-g block for Qbig expansion)
    pp = np.arange(128) // 8
    maskg = (pp[:, None] == pp[None, :]).astype(np.float32)
    return {"iota_f": iota_f, "iota_b": iota_b, "ident_f": ident_f,
            "maskg": maskg}


# ---- bass kernel builder ----------------------------------------------------

def build_nc(cfg, TA, TB):
    import concourse.bass as bass
    import concourse.bacc as bacc
    import concourse.mybir as mybir
    import concourse.tile as tile

    fp32 = mybir.dt.float32
    bf16 = mybir.dt.bfloat16
    i16 = mybir.dt.int16
    AX = mybir.AxisListType.X
    OP = mybir.AluOpType
    AF = mybir.ActivationFunctionType

    nb, npc, half, tg = cfg.nb, cfg.npc, cfg.half, cfg.tg
    totTA, totTB = int(np.sum(TA)), int(np.sum(TB))
    T_tot = totTA + totTB
    cumA = np.concatenate([[0], np.cumsum(TA)]).astype(int)
    cumB = np.concatenate([[0], np.cumsum(TB)]).astype(int)
    last_nn = npc - (nb - 1) * 128

    nc = bacc.Bacc("TRN2", target_bir_lowering=False, debug=False,
                   num_devices=cfg.ncores)

    # ---- dram I/O
    idx_d = nc.dram_tensor("idx", [128, T_tot * 8], i16, kind="ExternalInput")
    dstloc_d = nc.dram_tensor("dstloc", [128, T_tot], fp32, kind="ExternalInput")
    x_d = nc.dram_tensor("xq", [128, nb], fp32, kind="ExternalInput")
    lam_B0 = nc.dram_tensor("lamB0p", [128, M], fp32, kind="ExternalInput")
    lam_Pi = nc.dram_tensor("lamPip", [G, C], fp32, kind="ExternalInput")
    lam_Q = nc.dram_tensor("lamQp", [L - 1, 128, C], fp32, kind="ExternalInput")
    lam_B = nc.dram_tensor("lamBp", [L - 1, 128, M], fp32, kind="ExternalInput")
    pi_bounce = nc.dram_tensor("pi_bounce", [G * C], fp32)
    iota_f_d = nc.dram_tensor("iota_f", [128, 128], fp32, kind="ExternalInput")
    iota_b_d = nc.dram_tensor("iota_b", [128, 128], bf16, kind="ExternalInput")
    ident_f_d = nc.dram_tensor("ident_f", [128, 128], fp32, kind="ExternalInput")
    maskg_d = nc.dram_tensor("maskg", [128, 128], fp32, kind="ExternalInput")
    lls_d = nc.dram_tensor("lls", [npc, L * G], fp32, kind="ExternalOutput")

    lo_nb = cfg.lo_nb
    LO = lo_nb * 128
    HI = npc - LO
    h_slice_lo = [nc.dram_tensor(f"h_slo{l}", [LO, CG], bf16) for l in range(L - 1)]
    h_slice_hi = [nc.dram_tensor(f"h_shi{l}", [HI, CG], bf16) for l in range(L - 1)]
    h_full_lo = [nc.dram_tensor(f"h_flo{l}", [cfg.ncores * LO, CG], bf16,
                                addr_space="Shared") for l in range(L - 1)]
    h_full_hi = [nc.dram_tensor(f"h_fhi{l}", [cfg.ncores * HI, CG], bf16,
                                addr_space="Shared") for l in range(L - 1)]
    rgroups = [list(range(cfg.ncores))]
    nchA = -(-totTA // tg)
    nchB = -(-totTB // tg)
    ohA_dram = [nc.dram_tensor(f"ohA{ci}", [min(tg, totTA - ci * tg) * 128, 128],
                               bf16) for ci in range(nchA)]
    ohB_dram = [nc.dram_tensor(f"ohB{ci}", [min(tg, totTB - ci * tg) * 128, 128],
                               bf16) for ci in range(nchB)]

    with tile.TileContext(nc) as tc:
        from contextlib import ExitStack
        with ExitStack() as ctx:
            res = ctx.enter_context(tc.tile_pool(name="res", bufs=1))
            sbp = ctx.enter_context(tc.tile_pool(name="sbp", bufs=3))
            ohp = ctx.enter_context(tc.tile_pool(name="ohp", bufs=4))
            gpA = ctx.enter_context(tc.tile_pool(name="gpA", bufs=4))
            gpB = ctx.enter_context(tc.tile_pool(name="gpB", bufs=4))
            ohcp = ctx.enter_context(tc.tile_pool(name="ohcp", bufs=3))
            psp = ctx.enter_context(tc.tile_pool(name="psp", bufs=2, space="PSUM"))

            # ---- residents
            iota_f = res.tile([128, 128], fp32)
            nc.sync.dma_start(out=iota_f[:], in_=iota_f_d[:])
            iota_b = res.tile([128, 128], bf16)
            nc.sync.dma_start(out=iota_b[:], in_=iota_b_d[:])
            ident_f = res.tile([128, 128], fp32)
            nc.sync.dma_start(out=ident_f[:], in_=ident_f_d[:])
            maskg = res.tile([128, 128], fp32)
            nc.sync.dma_start(out=maskg[:], in_=maskg_d[:])
            idx_t = res.tile([128, T_tot * 8], i16)
            nc.sync.dma_start(out=idx_t[:], in_=idx_d[:])
            dstloc = res.tile([128, T_tot], fp32)
            nc.sync.dma_start(out=dstloc[:], in_=dstloc_d[:])
            x_t = res.tile([128, nb], fp32)
            nc.sync.dma_start(out=x_t[:], in_=x_d[:])
            ohXT = res.tile([32, nb * 128], fp32)     # one-hot(x)^T, all blocks
            out_sb = res.tile([128, nb * 64], fp32)   # lls accumulator
            qbig = res.tile([128, 128], fp32)
            barrT = res.tile([32, 128], fp32)         # layer's B table [m, cg]
            pi_col = res.tile([128, 1], fp32)

            def softmax_free(raw, nfree, tag):
                """softmax over free dim of raw [128p, nfree] fp32 -> new tile"""
                mx = sbp.tile([raw.shape[0], 1], fp32, tag=f"{tag}mx")
                nc.vector.tensor_reduce(out=mx[:], in_=raw[:], axis=AX,
                                        op=OP.max, negate=True)
                ex = sbp.tile([raw.shape[0], nfree], fp32, tag=f"{tag}ex")
                nc.scalar.activation(out=ex[:], in_=raw[:], func=AF.Exp,
                                     bias=mx[:, 0:1], scale=1.0)
                sm = sbp.tile([raw.shape[0], 1], fp32, tag=f"{tag}sm")
                nc.vector.reduce_sum(out=sm[:], in_=ex[:], axis=AX)
                rs = sbp.tile([raw.shape[0], 1], fp32, tag=f"{tag}rs")
                nc.vector.reciprocal(out=rs[:], in_=sm[:])
                out = sbp.tile([raw.shape[0], nfree], fp32, tag=f"{tag}out")
                nc.vector.tensor_scalar(out=out[:], in0=ex[:], scalar1=rs[:, 0:1],
                                        scalar2=None, op0=OP.mult)
                return out

            def prep_BarrT(src_ap, dest):
                """lambda_B-like [C, M, G] -> dest [32, 128] fp32 = B^T[m, (g c)],
                softmax over M; optionally scaled by pi_col."""
                raw = sbp.tile([128, M], fp32, tag="braw")
                nc.sync.dma_start(out=raw[:], in_=src_ap)
                bsm = softmax_free(raw, M, "b")
                return bsm

            def transpose_to(dest_sb, src_sb, pdim, fdim):
                """dest_sb [fdim, pdim] <- src_sb [pdim, fdim]^T via PE"""
                ps = psp.tile([fdim, pdim], fp32, tag="trp", space="PSUM")
                nc.tensor.transpose(out=ps[:], in_=src_sb[:],
                                    identity=ident_f[:pdim, :pdim])
                nc.scalar.copy(out=dest_sb[:], in_=ps[:])

            # ================= layer 0 =================
            # B0P[cg, m] = softmax_M(lambda_B0)[c,m,g] * Pi[c,g];  [(g c), m]
            b0sm = prep_BarrT(lam_B0[:], None)
            # Pi: [16, 8] softmax over free c, then scatter to [128, 1]
            praw = sbp.tile([16, C], fp32, tag="praw")
            nc.sync.dma_start(out=praw[:], in_=lam_Pi[:])
            pism = softmax_free(praw, C, "p")
            nc.sync.dma_start(out=pi_bounce[:].rearrange("(g c) -> g c", c=C),
                              in_=pism[:])
            nc.sync.dma_start(out=pi_col[:], in_=pi_bounce[:, None])
            b0p = sbp.tile([128, M], fp32, tag="b0p")
            nc.vector.tensor_scalar(out=b0p[:], in0=b0sm[:], scalar1=pi_col[:, 0:1],
                                    scalar2=None, op0=OP.mult)
            transpose_to(barrT, b0p, 128, 32)  # barrT <- B0P^T [m=32, cg]

            for b in range(nb):
                nn = 128 if b < nb - 1 else last_nn
                oh32 = sbp.tile([128, 32], fp32, tag="oh32")
                nc.vector.tensor_scalar(out=oh32[:], in0=iota_f[:, :32],
                                        scalar1=x_t[:, b:b + 1], scalar2=None,
                                        op0=OP.is_equal)
                trp = psp.tile([32, 128], fp32, tag="trp", space="PSUM")
                nc.tensor.transpose(out=trp[:], in_=oh32[:], identity=ident_f[:])
                nc.scalar.copy(out=ohXT[:, b * 128:(b + 1) * 128], in_=trp[:])
                u0p = psp.tile([128, 128], fp32, tag="bx", space="PSUM")
                nc.tensor.matmul(out=u0p[:], lhsT=ohXT[:, b * 128:(b + 1) * 128],
                                 rhs=barrT[:], start=True, stop=True)
                u = sbp.tile([128, 128], fp32, tag="u")
                nc.scalar.copy(out=u[:], in_=u0p[:])
                Z = sbp.tile([128, G], fp32, tag="Z")
                nc.vector.reduce_sum(out=Z[:], in_=u[:].rearrange(
                    "p (g c) -> p g c", c=C), axis=AX)
                nc.scalar.activation(out=out_sb[:, b * 64:b * 64 + G], in_=Z[:],
                                     func=AF.Ln)
                rz = sbp.tile([128, G], fp32, tag="rz")
                nc.vector.reciprocal(out=rz[:], in_=Z[:])
                h = sbp.tile([128, 128], bf16, tag="h")
                nc.vector.tensor_tensor(
                    out=h[:].rearrange("p (g c) -> p g c", c=C),
                    in0=u[:].rearrange("p (g c) -> p g c", c=C),
                    in1=rz[:].to_broadcast([128, G, C]), op=OP.mult)
                if b < lo_nb:
                    nc.sync.dma_start(out=h_slice_lo[0][b * 128:b * 128 + nn, :],
                                      in_=h[:nn, :])
                else:
                    bo = b - lo_nb
                    nc.sync.dma_start(out=h_slice_hi[0][bo * 128:bo * 128 + nn, :],
                                      in_=h[:nn, :])
                if b == lo_nb - 1:
                    nc.gpsimd.collective_compute(
                        "AllGather", OP.bypass, replica_groups=rgroups,
                        ins=[h_slice_lo[0][:]], outs=[h_full_lo[0][:]])

            # ---- prebuild one-hot tiles to DRAM (interleaved A/B chunk order)
            for ci in range(max(nchA, nchB)):
                for stream, nch, tot, dram in ((0, nchA, totTA, ohA_dram),
                                               (1, nchB, totTB, ohB_dram)):
                    if ci >= nch:
                        continue
                    colb = 0 if stream == 0 else totTA
                    ntile = min(tg, tot - ci * tg)
                    for t0 in range(0, ntile, 8):
                        nt8 = min(8, ntile - t0)
                        ohw = ohp.tile([128, 8 * 128], bf16, tag="ohw")
                        for j in range(nt8):
                            gt = ci * tg + t0 + j
                            nc.vector.tensor_scalar(
                                out=ohw[:, j * 128:(j + 1) * 128],
                                in0=iota_b[:],
                                scalar1=dstloc[:, colb + gt:colb + gt + 1],
                                scalar2=None, op0=OP.is_equal)
                        nc.sync.dma_start(
                            out=dram[ci][(t0) * 128:(t0 + nt8) * 128, :].rearrange(
                                "(t p) d -> p t d", p=128),
                            in_=ohw[:, :nt8 * 128].rearrange(
                                "p (t d) -> p t d", d=128))

            # ================= graph layers =================
            for l in range(1, L):
                lq = l - 1

                # ---- layer params
                qraw = sbp.tile([128, C], fp32, tag="qraw")
                nc.sync.dma_start(out=qraw[:], in_=lam_Q[lq])
                qsm = softmax_free(qraw, C, "q")  # [(g k), c]
                qsm_ap = qsm[:]
                qsm_bc = bass.AP(qsm_ap.tensor, qsm_ap.offset,
                                 [qsm_ap.ap[0], [0, G], qsm_ap.ap[1]])
                nc.vector.tensor_tensor(
                    out=qbig[:].rearrange("p (g c) -> p g c", c=C),
                    in0=qsm_bc,
                    in1=maskg[:].rearrange("p (g c) -> p g c", c=C),
                    op=OP.mult)
                bsm = prep_BarrT(lam_B[lq], None)
                transpose_to(barrT, bsm, 128, 32)

                # ---- gather + onehot chunk management
                chunk_cache = [{}, {}]
                oh_cache = [{}, {}]

                def get_oh(stream, t_idx):
                    tot = totTA if stream == 0 else totTB
                    dram = ohA_dram if stream == 0 else ohB_dram
                    cache = oh_cache[stream]
                    ci = t_idx // tg
                    if ci not in cache:
                        ntile = min(tg, tot - ci * tg)
                        buf = ohcp.tile([128, ntile * 128], bf16,
                                        tag=f"ohc{stream}")
                        nc.sync.dma_start(
                            out=buf[:].rearrange("p (t d) -> p t d", d=128),
                            in_=dram[ci][:].rearrange("(t p) d -> p t d", p=128))
                        cache[ci] = buf
                    return cache[ci][:].rearrange("p (t d) -> p t d", d=128)[
                        :, t_idx - ci * tg, :]

                def get_tile(stream, t_idx, l=l, lq=lq):
                    pool = gpA if stream == 0 else gpB
                    tot = totTA if stream == 0 else totTB
                    tab = h_full_lo[lq][:] if stream == 0 else h_full_hi[lq][:]
                    colb = 0 if stream == 0 else totTA * 8
                    cache = chunk_cache[stream]
                    ci = t_idx // tg
                    if ci not in cache:
                        ntile = min(tg, tot - ci * tg)
                        buf = pool.tile([128, ntile * 128], bf16,
                                        tag=f"g{stream}")
                        nc.gpsimd.dma_gather(
                            out_ap=buf[:].rearrange("p (t e) -> p t e", e=128),
                            in_ap=tab,
                            idxs_ap=idx_t[:, colb + ci * tg * 8:
                                          colb + (ci * tg + ntile) * 8],
                            num_idxs=ntile * 128,
                            num_idxs_reg=ntile * 128,
                            elem_size=128,
                            single_packet=False)
                        cache[ci] = buf
                    return cache[ci][:].rearrange("p (t e) -> p t e", e=128)[
                        :, t_idx - ci * tg, :]

                for ci in range(min(3, nchA)):
                    get_tile(0, ci * tg)
                    get_oh(0, ci * tg)
                # hi-bank AG for previous h (lo AG was traced mid-prev-layer);
                # traced after the lo prefetch so Pool starts desc-gen early
                nc.gpsimd.collective_compute(
                    "AllGather", OP.bypass, replica_groups=rgroups,
                    ins=[h_slice_hi[lq][:]], outs=[h_full_hi[lq][:]])

                for b in range(nb):
                    nn = 128 if b < nb - 1 else last_nn
                    agg = psp.tile([128, 128], fp32, tag="agg", space="PSUM")
                    nt = int(TA[b] + TB[b])
                    i = 0
                    for stream, cum in ((0, cumA), (1, cumB)):
                        Tb = int(TA[b] if stream == 0 else TB[b])
                        colb = 0 if stream == 0 else totTA
                        for t in range(Tb):
                            gt = int(cum[b]) + t
                            gat = get_tile(stream, gt)
                            oh = get_oh(stream, gt)
                            nc.tensor.matmul(out=agg[:], lhsT=oh, rhs=gat,
                                             start=(i == 0), stop=(i == nt - 1))
                            i += 1

                    aggsb = sbp.tile([128, 128], fp32, tag="aggsb")
                    nc.scalar.copy(out=aggsb[:], in_=agg[:])
                    cnt = sbp.tile([128, 1], fp32, tag="cnt")
                    nc.vector.reduce_sum(out=cnt[:], in_=aggsb[:], axis=AX)
                    logcnt = sbp.tile([128, 1], fp32, tag="logcnt")
                    nc.scalar.activation(out=logcnt[:], in_=cnt[:], func=AF.Ln,
                                         scale=1.0 / G)
                    # QA^T = Qbig^T(lhsT=qbig) @ aggr^T
                    trp = psp.tile([128, 128], fp32, tag="trp", space="PSUM")
                    nc.tensor.transpose(out=trp[:], in_=aggsb[:],
                                        identity=ident_f[:])
                    aggT = sbp.tile([128, 128], fp32, tag="aggT")
                    nc.scalar.copy(out=aggT[:], in_=trp[:])
                    qaT = psp.tile([128, 128], fp32, tag="qa", space="PSUM")
                    nc.tensor.matmul(out=qaT[:], lhsT=qbig[:], rhs=aggT[:],
                                     start=True, stop=True)
                    qaTsb = sbp.tile([128, 128], fp32, tag="qaTsb")
                    nc.scalar.copy(out=qaTsb[:], in_=qaT[:])
                    qa2 = psp.tile([128, 128], fp32, tag="trp", space="PSUM")
                    nc.tensor.transpose(out=qa2[:], in_=qaTsb[:],
                                        identity=ident_f[:])
                    bx = psp.tile([128, 128], fp32, tag="bx", space="PSUM")
                    nc.tensor.matmul(out=bx[:],
                                     lhsT=ohXT[:, b * 128:(b + 1) * 128],
                                     rhs=barrT[:], start=True, stop=True)
                    bxsb = sbp.tile([128, 128], fp32, tag="bxsb")
                    nc.scalar.copy(out=bxsb[:], in_=bx[:])
                    u = sbp.tile([128, 128], fp32, tag="u")
                    nc.vector.tensor_tensor(out=u[:], in0=qa2[:], in1=bxsb[:],
                                            op=OP.mult)
                    Z = sbp.tile([128, G], fp32, tag="Z")
                    nc.vector.reduce_sum(out=Z[:], in_=u[:].rearrange(
                        "p (g c) -> p g c", c=C), axis=AX)
                    logZ = sbp.tile([128, G], fp32, tag="logZ")
                    nc.scalar.activation(out=logZ[:], in_=Z[:], func=AF.Ln)
                    nc.vector.tensor_scalar(
                        out=out_sb[:, b * 64 + l * G:b * 64 + (l + 1) * G],
                        in0=logZ[:], scalar1=logcnt[:, 0:1], scalar2=None,
                        op0=OP.subtract)
                    if l < L - 1:
                        rz = sbp.tile([128, G], fp32, tag="rz")
                        nc.vector.reciprocal(out=rz[:], in_=Z[:])
                        h = sbp.tile([128, 128], bf16, tag="h")
                        nc.vector.tensor_tensor(
                            out=h[:].rearrange("p (g c) -> p g c", c=C),
                            in0=u[:].rearrange("p (g c) -> p g c", c=C),
                            in1=rz[:].to_broadcast([128, G, C]), op=OP.mult)
                        if b < lo_nb:
                            nc.sync.dma_start(
                                out=h_slice_lo[l][b * 128:b * 128 + nn, :],
                                in_=h[:nn, :])
                        else:
                            bo = b - lo_nb
                            nc.sync.dma_start(
                                out=h_slice_hi[l][bo * 128:bo * 128 + nn, :],
                                in_=h[:nn, :])
                        if b == lo_nb - 1:
                            nc.gpsimd.collective_compute(
                                "AllGather", OP.bypass, replica_groups=rgroups,
                                ins=[h_slice_lo[l][:]], outs=[h_full_lo[l][:]])

            # ---- write lls out
            if nb > 1:
                nc.sync.dma_start(
                    out=lls_d[:(nb - 1) * 128, :].rearrange(
                        "(b p) c -> p b c", p=128),
                    in_=out_sb[:].rearrange("p (b c) -> p b c", c=64)[:, :nb - 1, :])
            nc.sync.dma_start(
                out=lls_d[(nb - 1) * 128:, :],
                in_=out_sb[:last_nn, (nb - 1) * 64:nb * 64])

    nc.compile()
    return nc


# ---- entry point ------------------------------------------------------------

def kernel(x, edge_index, lambda_B0, lambda_Pi, lambda_Q, lambda_B):
    cfg = Cfg()
    cores, TA, TB = preprocess(x, edge_index, cfg)
    consts = make_consts()
    nc = build_nc(cfg, TA, TB)

    from concourse.bass_utils import run_bass_kernel_spmd
    params = permute_params(lambda_B0, lambda_Pi, lambda_Q, lambda_B)
    in_maps = []
    for c in range(cfg.ncores):
        m = dict(cores[c])
        m.update(params)
        m.update({k: np.ascontiguousarray(v) for k, v in consts.items()})
        in_maps.append(m)

    res = run_bass_kernel_spmd(nc, in_maps, core_ids=list(range(cfg.ncores)))
    out = np.concatenate([res.results[c]["lls"] for c in range(cfg.ncores)],
                         axis=0)
    return out.reshape(N, L, G).astype(np.float32)



# revision 2
# speedup vs baseline: 1.5217x; 1.5217x over previous
"""CGMM (Contextual Graph Markov Model) forward pass on 8 Trainium2 NeuronCores.

Self-contained: takes FULL inputs as numpy arrays, shards nodes/edges across
the 8 cores (graph parallel), runs a Bass/Tile kernel via
run_bass_kernel_spmd, returns the FULL [N, L, G] log-likelihood output.

Key layout (per core, nodes on partitions, cg = g*8 + c on free dim):
  layer 0:  u0[n, cg] = B0[c, x_n, g]*Pi[c, g]  via host-built one-hot(x) matmul
  layers 1..3:
            h split into NBANKS node-range banks; AllGather per bank overlaps
            the previous bank's compute (pipelined collectives)
            gather h_full_bank[src] per edge via dma_gather spread ROUND-ROBIN
            over 4 SWDGE queues (4 Q7 core pairs emit descriptors in parallel)
            aggT[cg, dst] = segment-sum via host-built one-hot matmuls
            (lhsT=gathered, rhs=onehot -> transposed aggregate, PSUM fp32),
            accumulated across banks in an SBUF fp32 tile
            QA^T = Qbig @ aggT; u = Bx * QA; Z = sum_c u; ll = log Z; h = u/Z
Host precomputes: edge sort/tiling, one-hot tiles (bf16), one-hot(x),
in-degree log-counts (applied as output post-processing: ll -= log cnt).
"""
import sys

sys.path.insert(0, "/opt/trn_rl_repo")

import numpy as np
import ml_dtypes

BF = ml_dtypes.bfloat16

# ---- problem sizes (hardcoded per contract) --------------------------------
N, E, C, M, G, L = 50000, 800000, 8, 32, 16, 4
NCORES = 8
CG = C * G  # 128
NBANKS = 3  # h banks (pipelined AllGather)
NQ = 4      # SWDGE queues used for dma_gather
TG = 16     # gather chunk size in 128-edge tiles


def split_blocks(nb, k):
    base = nb // k
    rem = nb % k
    return [base + (1 if i < rem else 0) for i in range(k)]


class Cfg:
    def __init__(self, n=N, e=E, ncores=NCORES):
        self.n = n
        self.e = e
        self.ncores = ncores
        self.npc = n // ncores
        self.nb = (self.npc + 127) // 128
        self.last_nn = self.npc - (self.nb - 1) * 128
        self.bank_blocks = split_blocks(self.nb, NBANKS)
        self.bank_first = np.concatenate([[0], np.cumsum(self.bank_blocks)])
        # nodes per bank within one core (last bank absorbs the short block)
        self.bank_node_start = [int(self.bank_first[k]) * 128
                                for k in range(NBANKS)]
        self.bank_nodes = [
            (int(self.bank_first[k + 1]) * 128 if k < NBANKS - 1 else self.npc)
            - self.bank_node_start[k]
            for k in range(NBANKS)]
        self.bank_of_block = np.searchsorted(
            self.bank_first[1:], np.arange(self.nb), side="right")


# ---- host preprocessing -----------------------------------------------------

def preprocess(x, edge_index, cfg):
    """Edge sort + tile schedule + one-hot tiles + degree counts (host)."""
    dst = np.asarray(edge_index[0], dtype=np.int64)
    src = np.asarray(edge_index[1], dtype=np.int64)
    x = np.asarray(x, dtype=np.int64)
    nc_, npc, nb = cfg.ncores, cfg.npc, cfg.nb

    owner = dst // npc
    per_core = []
    cnts = np.zeros((nc_, nb, NBANKS), dtype=np.int64)
    lncnt = np.zeros(cfg.n, dtype=np.float32)
    bob = cfg.bank_of_block
    bns = np.asarray(cfg.bank_node_start, dtype=np.int64)
    bsz = np.asarray(cfg.bank_nodes, dtype=np.int64)
    for c in range(nc_):
        sel = owner == c
        d = dst[sel] - c * npc
        s = src[sel]
        deg = np.bincount(d, minlength=npc)
        lncnt[c * npc:(c + 1) * npc] = np.log(np.maximum(deg, 1))
        b = d // 128
        dl = d % 128
        sown = s // npc
        soff = s % npc
        kb = bob[soff // 128]
        row = sown * bsz[kb] + (soff - bns[kb])
        per_core.append((b, dl, row, kb))
        key = b * NBANKS + kb
        cnts[c] = np.bincount(key, minlength=nb * NBANKS).reshape(nb, NBANKS)
    T = np.maximum(1, -(-cnts.max(axis=0) // 128))  # [nb, NBANKS]

    # tile offsets in (bank-major, block-minor) order
    off = np.zeros((NBANKS, nb), dtype=np.int64)
    pos = 0
    bank_t0 = []
    for k in range(NBANKS):
        bank_t0.append(pos)
        for b in range(nb):
            off[k, b] = pos
            pos += int(T[b, k])
    T_tot = pos
    bank_t1 = bank_t0[1:] + [T_tot]

    cores = []
    for c in range(nc_):
        b, dl, row, kb = per_core[c]
        idxs = np.zeros(T_tot * 128, dtype=np.int64)
        dls = np.full(T_tot * 128, -1, dtype=np.int64)
        order = np.argsort(kb * nb * 64 + b, kind="stable")
        b, dl, row, kb = b[order], dl[order], row[order], kb[order]
        # group boundaries: edges sorted by (bank, block)
        grp = kb * nb + b
        starts = np.searchsorted(grp, np.arange(NBANKS * nb), side="left")
        ends = np.searchsorted(grp, np.arange(NBANKS * nb), side="right")
        for k in range(NBANKS):
            for bb in range(nb):
                g0, g1 = starts[k * nb + bb], ends[k * nb + bb]
                if g1 <= g0:
                    continue
                a0 = off[k, bb] * 128
                idxs[a0:a0 + (g1 - g0)] = row[g0:g1]
                dls[a0:a0 + (g1 - g0)] = dl[g0:g1]

        # idx dram layout: [128, T_tot*8] int16; 16-row wrap, replicated 8x
        idx16 = idxs.astype(np.int16).reshape(-1, 16).T  # [16, T_tot*8]
        idx_d = np.tile(idx16, (8, 1))                   # [128, T_tot*8]

        # one-hot tiles: ohx[p, t*128 + d] = 1 iff dls[t*128+p] == d
        ohx = np.zeros((128, T_tot * 128), dtype=BF)
        i_all = np.arange(T_tot * 128)
        m = dls >= 0
        ohx[i_all[m] % 128, (i_all[m] // 128) * 128 + dls[m]] = 1

        # one-hot(x)^T: [32, nb*128] fp32
        xloc = np.zeros(nb * 128, dtype=np.int64)
        xloc[:npc] = x[c * npc:(c + 1) * npc]
        ohxt = np.zeros((M, nb * 128), dtype=np.float32)
        ohxt[xloc, np.arange(nb * 128)] = 1
        ohxt[:, npc:] = 0  # padded tail nodes: no contribution needed anyway

        cores.append({"idx": np.ascontiguousarray(idx_d),
                      "ohx": np.ascontiguousarray(ohx),
                      "ohxt": np.ascontiguousarray(ohxt)})
    sched = {"T": T, "off": off, "T_tot": T_tot,
             "bank_t0": bank_t0, "bank_t1": bank_t1}
    return cores, sched, lncnt


def permute_params(lambda_B0, lambda_Pi, lambda_Q, lambda_B):
    """Pure layout permutations (no compute): partition (g, c/k)-major views."""
    lamB0p = np.ascontiguousarray(
        np.transpose(np.asarray(lambda_B0, np.float32), (2, 0, 1)).reshape(G * C, M))
    lamPip = np.ascontiguousarray(np.asarray(lambda_Pi, np.float32).T)  # [G, C]
    lamQp = np.ascontiguousarray(
        np.transpose(np.asarray(lambda_Q, np.float32), (0, 3, 2, 1)).reshape(
            L - 1, G * C, C))
    lamBp = np.ascontiguousarray(
        np.transpose(np.asarray(lambda_B, np.float32), (0, 3, 1, 2)).reshape(
            L - 1, G * C, M))
    return {"lamB0p": lamB0p, "lamPip": lamPip, "lamQp": lamQp, "lamBp": lamBp}


def make_consts():
    ident_f = np.eye(128, dtype=np.float32)
    pp = np.arange(128) // 8
    maskg = (pp[:, None] == pp[None, :]).astype(np.float32)
    return {"ident_f": ident_f, "maskg": maskg}


# ---- bass kernel builder ----------------------------------------------------

def build_nc(cfg, sched):
    import concourse.bass as bass
    import concourse.bacc as bacc
    import concourse.mybir as mybir
    import concourse.tile as tile

    fp32 = mybir.dt.float32
    bf16 = mybir.dt.bfloat16
    i16 = mybir.dt.int16
    AX = mybir.AxisListType.X
    OP = mybir.AluOpType
    AF = mybir.ActivationFunctionType

    nb, npc, last_nn = cfg.nb, cfg.npc, cfg.last_nn
    T, off, T_tot = sched["T"], sched["off"], sched["T_tot"]
    bank_t0, bank_t1 = sched["bank_t0"], sched["bank_t1"]

    nc = bacc.Bacc("TRN2", target_bir_lowering=False, debug=False,
                   num_devices=cfg.ncores, num_swdge_queues=NQ)

    # ---- dram I/O
    idx_d = nc.dram_tensor("idx", [128, T_tot * 8], i16, kind="ExternalInput")
    ohx_d = nc.dram_tensor("ohx", [128, T_tot * 128], bf16, kind="ExternalInput")
    ohxt_d = nc.dram_tensor("ohxt", [M, nb * 128], fp32, kind="ExternalInput")
    lam_B0 = nc.dram_tensor("lamB0p", [128, M], fp32, kind="ExternalInput")
    lam_Pi = nc.dram_tensor("lamPip", [G, C], fp32, kind="ExternalInput")
    lam_Q = nc.dram_tensor("lamQp", [L - 1, 128, C], fp32, kind="ExternalInput")
    lam_B = nc.dram_tensor("lamBp", [L - 1, 128, M], fp32, kind="ExternalInput")
    pi_bounce = nc.dram_tensor("pi_bounce", [G * C], fp32)
    ident_f_d = nc.dram_tensor("ident_f", [128, 128], fp32, kind="ExternalInput")
    maskg_d = nc.dram_tensor("maskg", [128, 128], fp32, kind="ExternalInput")
    lls_d = nc.dram_tensor("lls", [npc, L * G], fp32, kind="ExternalOutput")

    h_slice = [[nc.dram_tensor(f"h_s{l}_{k}", [cfg.bank_nodes[k], CG], bf16)
                for k in range(NBANKS)] for l in range(L - 1)]
    h_full = [[nc.dram_tensor(f"h_f{l}_{k}", [cfg.ncores * cfg.bank_nodes[k], CG],
                              bf16, addr_space="Shared")
               for k in range(NBANKS)] for l in range(L - 1)]
    rgroups = [list(range(cfg.ncores))]

    # chunk table: list of (bank, t0, ntile); queue = index % NQ
    chunks = []
    tile2chunk = np.zeros(T_tot, dtype=np.int64)
    for k in range(NBANKS):
        for t0 in range(bank_t0[k], bank_t1[k], TG):
            ntile = min(TG, bank_t1[k] - t0)
            tile2chunk[t0:t0 + ntile] = len(chunks)
            chunks.append((k, t0, ntile))

    with tile.TileContext(nc) as tc:
        from contextlib import ExitStack
        with ExitStack() as ctx:
            res = ctx.enter_context(tc.tile_pool(name="res", bufs=1))
            sbp = ctx.enter_context(tc.tile_pool(name="sbp", bufs=3))
            gp = ctx.enter_context(tc.tile_pool(name="gp", bufs=2 * NQ))
            ohp = ctx.enter_context(tc.tile_pool(name="ohp", bufs=6))
            psp = ctx.enter_context(tc.tile_pool(name="psp", bufs=2, space="PSUM"))

            # ---- residents
            ident_f = res.tile([128, 128], fp32)
            nc.sync.dma_start(out=ident_f[:], in_=ident_f_d[:])
            maskg = res.tile([128, 128], fp32)
            nc.sync.dma_start(out=maskg[:], in_=maskg_d[:])
            idx_t = res.tile([128, T_tot * 8], i16)
            nc.sync.dma_start(out=idx_t[:], in_=idx_d[:])
            ohxt = res.tile([M, nb * 128], fp32)
            nc.sync.dma_start(out=ohxt[:], in_=ohxt_d[:])
            out_sb = res.tile([128, nb * 64], fp32)   # lls accumulator
            aggsb = res.tile([128, nb * 128], fp32)   # aggT accumulator
            qbig = res.tile([128, 128], fp32)
            barrT = res.tile([32, 128], fp32)         # layer's B table [m, cg]
            pi_col = res.tile([128, 1], fp32)

            def softmax_free(raw, nfree, tag):
                mx = sbp.tile([raw.shape[0], 1], fp32, tag=f"{tag}mx")
                nc.vector.tensor_reduce(out=mx[:], in_=raw[:], axis=AX,
                                        op=OP.max, negate=True)
                ex = sbp.tile([raw.shape[0], nfree], fp32, tag=f"{tag}ex")
                nc.scalar.activation(out=ex[:], in_=raw[:], func=AF.Exp,
                                     bias=mx[:, 0:1], scale=1.0)
                sm = sbp.tile([raw.shape[0], 1], fp32, tag=f"{tag}sm")
                nc.vector.reduce_sum(out=sm[:], in_=ex[:], axis=AX)
                rs = sbp.tile([raw.shape[0], 1], fp32, tag=f"{tag}rs")
                nc.vector.reciprocal(out=rs[:], in_=sm[:])
                out = sbp.tile([raw.shape[0], nfree], fp32, tag=f"{tag}out")
                nc.vector.tensor_scalar(out=out[:], in0=ex[:], scalar1=rs[:, 0:1],
                                        scalar2=None, op0=OP.mult)
                return out

            def transpose_to(dest_sb, src_sb, pdim, fdim):
                ps = psp.tile([fdim, pdim], fp32, tag="trp", space="PSUM")
                nc.tensor.transpose(out=ps[:], in_=src_sb[:],
                                    identity=ident_f[:pdim, :pdim])
                nc.scalar.copy(out=dest_sb[:], in_=ps[:])

            def bank_of_block(b):
                return int(cfg.bank_of_block[b])

            def maybe_ag(l, b):
                kh = bank_of_block(b)
                if b == int(cfg.bank_first[kh + 1]) - 1:
                    nc.gpsimd.collective_compute(
                        "AllGather", OP.bypass, replica_groups=rgroups,
                        ins=[h_slice[l][kh][:]], outs=[h_full[l][kh][:]])

            def emit_h(l, b, u, Z):
                """rz = 1/Z; h = u*rz (bf16); DMA to h_slice; fire AG at bank end."""
                rz = sbp.tile([128, G], fp32, tag="rz")
                nc.vector.reciprocal(out=rz[:], in_=Z[:])
                h = sbp.tile([128, 128], bf16, tag="h")
                nc.vector.tensor_tensor(
                    out=h[:].rearrange("p (g c) -> p g c", c=C),
                    in0=u[:].rearrange("p (g c) -> p g c", c=C),
                    in1=rz[:].to_broadcast([128, G, C]), op=OP.mult)
                kh = bank_of_block(b)
                r0 = (b - int(cfg.bank_first[kh])) * 128
                nn = 128 if b < nb - 1 else last_nn
                nc.sync.dma_start(out=h_slice[l][kh][r0:r0 + nn, :],
                                  in_=h[:nn, :])
                maybe_ag(l, b)

            # ================= layer 0 =================
            braw = sbp.tile([128, M], fp32, tag="braw")
            nc.sync.dma_start(out=braw[:], in_=lam_B0[:])
            b0sm = softmax_free(braw, M, "b")
            praw = sbp.tile([G, C], fp32, tag="praw")
            nc.sync.dma_start(out=praw[:], in_=lam_Pi[:])
            pism = softmax_free(praw, C, "p")
            nc.sync.dma_start(out=pi_bounce[:].rearrange("(g c) -> g c", c=C),
                              in_=pism[:])
            nc.sync.dma_start(out=pi_col[:], in_=pi_bounce[:, None])
            b0p = sbp.tile([128, M], fp32, tag="b0p")
            nc.vector.tensor_scalar(out=b0p[:], in0=b0sm[:], scalar1=pi_col[:, 0:1],
                                    scalar2=None, op0=OP.mult)
            transpose_to(barrT, b0p, 128, 32)

            for b in range(nb):
                u0p = psp.tile([128, 128], fp32, tag="bx", space="PSUM")
                nc.tensor.matmul(out=u0p[:], lhsT=ohxt[:, b * 128:(b + 1) * 128],
                                 rhs=barrT[:], start=True, stop=True)
                u = sbp.tile([128, 128], fp32, tag="u")
                nc.scalar.copy(out=u[:], in_=u0p[:])
                Z = sbp.tile([128, G], fp32, tag="Z")
                nc.vector.reduce_sum(out=Z[:], in_=u[:].rearrange(
                    "p (g c) -> p g c", c=C), axis=AX)
                nc.scalar.activation(out=out_sb[:, b * 64:b * 64 + G], in_=Z[:],
                                     func=AF.Ln)
                emit_h(0, b, u, Z)

            # ================= graph layers =================
            for l in range(1, L):
                lq = l - 1

                qraw = sbp.tile([128, C], fp32, tag="qraw")
                nc.sync.dma_start(out=qraw[:], in_=lam_Q[lq])
                qsm = softmax_free(qraw, C, "q")  # [(g k), c]
                qsm_ap = qsm[:]
                qsm_bc = bass.AP(qsm_ap.tensor, qsm_ap.offset,
                                 [qsm_ap.ap[0], [0, G], qsm_ap.ap[1]])
                nc.vector.tensor_tensor(
                    out=qbig[:].rearrange("p (g c) -> p g c", c=C),
                    in0=qsm_bc,
                    in1=maskg[:].rearrange("p (g c) -> p g c", c=C),
                    op=OP.mult)
                braw2 = sbp.tile([128, M], fp32, tag="braw")
                nc.sync.dma_start(out=braw2[:], in_=lam_B[lq])
                bsm = softmax_free(braw2, M, "b")
                transpose_to(barrT, bsm, 128, 32)

                # ---- chunk caches (fresh per layer)
                gat_cache = {}
                oh_cache = {}

                def get_gat(gt, lq=lq):
                    ci = int(tile2chunk[gt])
                    if ci not in gat_cache:
                        k, t0, ntile = chunks[ci]
                        buf = gp.tile([128, ntile * 128], bf16, tag="g")
                        nc.gpsimd.dma_gather(
                            out_ap=buf[:].rearrange("p (t e) -> p t e", e=128),
                            in_ap=h_full[lq][k][:],
                            idxs_ap=idx_t[:, t0 * 8:(t0 + ntile) * 8],
                            num_idxs=ntile * 128,
                            num_idxs_reg=ntile * 128,
                            elem_size=128,
                            single_packet=False,
                            queue_num=ci % NQ)
                        gat_cache[ci] = buf
                    k, t0, ntile = chunks[ci]
                    return gat_cache[ci][:].rearrange(
                        "p (t e) -> p t e", e=128)[:, gt - t0, :]

                def get_oh(gt):
                    ci = int(tile2chunk[gt])
                    if ci not in oh_cache:
                        k, t0, ntile = chunks[ci]
                        buf = ohp.tile([128, ntile * 128], bf16, tag="oh")
                        nc.sync.dma_start(
                            out=buf[:], in_=ohx_d[:, t0 * 128:(t0 + ntile) * 128])
                        oh_cache[ci] = buf
                    k, t0, ntile = chunks[ci]
                    return oh_cache[ci][:, (gt - t0) * 128:(gt - t0 + 1) * 128]

                def finish(b, l=l):
                    sl = aggsb[:, b * 128:(b + 1) * 128]
                    qaT = psp.tile([128, 128], fp32, tag="qa", space="PSUM")
                    nc.tensor.matmul(out=qaT[:], lhsT=qbig[:], rhs=sl,
                                     start=True, stop=True)
                    qaTsb = sbp.tile([128, 128], fp32, tag="qaTsb")
                    nc.scalar.copy(out=qaTsb[:], in_=qaT[:])
                    qa2 = psp.tile([128, 128], fp32, tag="trp", space="PSUM")
                    nc.tensor.transpose(out=qa2[:], in_=qaTsb[:],
                                        identity=ident_f[:])
                    bx = psp.tile([128, 128], fp32, tag="bx", space="PSUM")
                    nc.tensor.matmul(out=bx[:],
                                     lhsT=ohxt[:, b * 128:(b + 1) * 128],
                                     rhs=barrT[:], start=True, stop=True)
                    bxsb = sbp.tile([128, 128], fp32, tag="bxsb")
                    nc.scalar.copy(out=bxsb[:], in_=bx[:])
                    u = sbp.tile([128, 128], fp32, tag="u")
                    nc.vector.tensor_tensor(out=u[:], in0=qa2[:], in1=bxsb[:],
                                            op=OP.mult)
                    Z = sbp.tile([128, G], fp32, tag="Z")
                    nc.vector.reduce_sum(out=Z[:], in_=u[:].rearrange(
                        "p (g c) -> p g c", c=C), axis=AX)
                    nc.scalar.activation(
                        out=out_sb[:, b * 64 + l * G:b * 64 + (l + 1) * G],
                        in_=Z[:], func=AF.Ln)
                    if l < L - 1:
                        emit_h(l, b, u, Z)

                pending_finish = None
                for k in range(NBANKS):
                    for b in range(nb):
                        nt = int(T[b, k])
                        ps = psp.tile([128, 128], fp32, tag="agg", space="PSUM")
                        for t in range(nt):
                            gt = int(off[k, b]) + t
                            nc.tensor.matmul(out=ps[:], lhsT=get_gat(gt),
                                             rhs=get_oh(gt),
                                             start=(t == 0), stop=(t == nt - 1))
                        sl = aggsb[:, b * 128:(b + 1) * 128]
                        if k == 0:
                            nc.scalar.copy(out=sl, in_=ps[:])
                        else:
                            nc.vector.tensor_tensor(out=sl, in0=ps[:], in1=sl,
                                                    op=OP.add)
                        if k == NBANKS - 1:
                            if pending_finish is not None:
                                finish(pending_finish)
                            pending_finish = b
                if pending_finish is not None:
                    finish(pending_finish)

            # ---- write lls out
            if nb > 1:
                nc.sync.dma_start(
                    out=lls_d[:(nb - 1) * 128, :].rearrange(
                        "(b p) c -> p b c", p=128),
                    in_=out_sb[:].rearrange("p (b c) -> p b c", c=64)[:, :nb - 1, :])
            nc.sync.dma_start(
                out=lls_d[(nb - 1) * 128:, :],
                in_=out_sb[:last_nn, (nb - 1) * 64:nb * 64])

    nc.compile()
    return nc


# ---- entry point ------------------------------------------------------------

def kernel(x, edge_index, lambda_B0, lambda_Pi, lambda_Q, lambda_B):
    cfg = Cfg()
    cores, sched, lncnt = preprocess(x, edge_index, cfg)
    consts = make_consts()
    nc = build_nc(cfg, sched)

    from concourse.bass_utils import run_bass_kernel_spmd
    params = permute_params(lambda_B0, lambda_Pi, lambda_Q, lambda_B)
    in_maps = []
    for c in range(cfg.ncores):
        m = dict(cores[c])
        m.update(params)
        m.update({k: np.ascontiguousarray(v) for k, v in consts.items()})
        in_maps.append(m)

    res = run_bass_kernel_spmd(nc, in_maps, core_ids=list(range(cfg.ncores)))
    out = np.concatenate([res.results[c]["lls"] for c in range(cfg.ncores)],
                         axis=0).reshape(N, L, G).astype(np.float32)
    out[:, 1:, :] -= lncnt[:, None, None]
    return out


# revision 11
# speedup vs baseline: 1.5340x; 1.0081x over previous
"""CGMM (Contextual Graph Markov Model) forward pass on 8 Trainium2 NeuronCores.

Self-contained: takes FULL inputs as numpy arrays, shards nodes/edges across
the 8 cores (graph parallel), runs a Bass/Tile kernel via
run_bass_kernel_spmd, returns the FULL [N, L, G] log-likelihood output.

Key layout (per core, nodes on partitions, cg = g*8 + c on free dim):
  layer 0:  u0[n, cg] = B0[c, x_n, g]*Pi[c, g]  via host-built one-hot(x) matmul
  layers 1..3:
            h split into NBANKS node-range banks; AllGather per bank overlaps
            the previous bank's compute (pipelined collectives)
            gather h_full_bank[src] per edge via dma_gather spread ROUND-ROBIN
            over 4 SWDGE queues (4 Q7 core pairs emit descriptors in parallel)
            aggT[cg, dst] = segment-sum via host-built one-hot matmuls
            (lhsT=gathered, rhs=onehot -> transposed aggregate, PSUM fp32),
            accumulated across banks in an SBUF fp32 tile
            QA^T = Qbig @ aggT; u = Bx * QA; Z = sum_c u; ll = log Z; h = u/Z
Host precomputes: edge sort/tiling, one-hot tiles (bf16), one-hot(x),
in-degree log-counts (applied as output post-processing: ll -= log cnt).
"""
import sys

sys.path.insert(0, "/opt/trn_rl_repo")

import numpy as np
import ml_dtypes

BF = ml_dtypes.bfloat16

# ---- problem sizes (hardcoded per contract) --------------------------------
N, E, C, M, G, L = 50000, 800000, 8, 32, 16, 4
NCORES = 8
CG = C * G  # 128
NBANKS = 3  # h banks (pipelined AllGather)
NQ = 4      # SWDGE queues used for dma_gather
TG = 16     # gather chunk size in 128-edge tiles


def split_blocks(nb, k):
    base = nb // k
    rem = nb % k
    return [base + (1 if i < rem else 0) for i in range(k)]


class Cfg:
    def __init__(self, n=N, e=E, ncores=NCORES):
        self.n = n
        self.e = e
        self.ncores = ncores
        self.npc = n // ncores
        self.nb = (self.npc + 127) // 128
        self.last_nn = self.npc - (self.nb - 1) * 128
        self.bank_blocks = split_blocks(self.nb, NBANKS)
        self.bank_first = np.concatenate([[0], np.cumsum(self.bank_blocks)])
        # nodes per bank within one core (last bank absorbs the short block)
        self.bank_node_start = [int(self.bank_first[k]) * 128
                                for k in range(NBANKS)]
        self.bank_nodes = [
            (int(self.bank_first[k + 1]) * 128 if k < NBANKS - 1 else self.npc)
            - self.bank_node_start[k]
            for k in range(NBANKS)]
        self.bank_of_block = np.searchsorted(
            self.bank_first[1:], np.arange(self.nb), side="right")


# ---- host preprocessing -----------------------------------------------------

def preprocess(x, edge_index, cfg):
    """Edge sort + tile schedule + one-hot tiles + degree counts (host)."""
    dst = np.asarray(edge_index[0], dtype=np.int64)
    src = np.asarray(edge_index[1], dtype=np.int64)
    x = np.asarray(x, dtype=np.int64)
    nc_, npc, nb = cfg.ncores, cfg.npc, cfg.nb

    owner = dst // npc
    per_core = []
    cnts = np.zeros((nc_, nb, NBANKS), dtype=np.int64)
    lncnt = np.zeros(cfg.n, dtype=np.float32)
    bob = cfg.bank_of_block
    bns = np.asarray(cfg.bank_node_start, dtype=np.int64)
    bsz = np.asarray(cfg.bank_nodes, dtype=np.int64)
    for c in range(nc_):
        sel = owner == c
        d = dst[sel] - c * npc
        s = src[sel]
        deg = np.bincount(d, minlength=npc)
        lncnt[c * npc:(c + 1) * npc] = np.log(np.maximum(deg, 1))
        b = d // 128
        dl = d % 128
        sown = s // npc
        soff = s % npc
        kb = bob[soff // 128]
        row = sown * bsz[kb] + (soff - bns[kb])
        per_core.append((b, dl, row, kb))
        key = b * NBANKS + kb
        cnts[c] = np.bincount(key, minlength=nb * NBANKS).reshape(nb, NBANKS)
    T = np.maximum(1, -(-cnts.max(axis=0) // 128))  # [nb, NBANKS]

    # tile offsets in (bank-major, block-minor) order
    off = np.zeros((NBANKS, nb), dtype=np.int64)
    pos = 0
    bank_t0 = []
    for k in range(NBANKS):
        bank_t0.append(pos)
        for b in range(nb):
            off[k, b] = pos
            pos += int(T[b, k])
    T_tot = pos
    bank_t1 = bank_t0[1:] + [T_tot]

    cores = []
    for c in range(nc_):
        b, dl, row, kb = per_core[c]
        idxs = np.zeros(T_tot * 128, dtype=np.int64)
        dls = np.full(T_tot * 128, -1, dtype=np.int64)
        order = np.argsort(kb * nb * 64 + b, kind="stable")
        b, dl, row, kb = b[order], dl[order], row[order], kb[order]
        # group boundaries: edges sorted by (bank, block)
        grp = kb * nb + b
        starts = np.searchsorted(grp, np.arange(NBANKS * nb), side="left")
        ends = np.searchsorted(grp, np.arange(NBANKS * nb), side="right")
        for k in range(NBANKS):
            for bb in range(nb):
                g0, g1 = starts[k * nb + bb], ends[k * nb + bb]
                if g1 <= g0:
                    continue
                a0 = off[k, bb] * 128
                idxs[a0:a0 + (g1 - g0)] = row[g0:g1]
                dls[a0:a0 + (g1 - g0)] = dl[g0:g1]

        # idx dram layout: [128, T_tot*8] int16; 16-row wrap, replicated 8x
        idx16 = idxs.astype(np.int16).reshape(-1, 16).T  # [16, T_tot*8]
        idx_d = np.tile(idx16, (8, 1))                   # [128, T_tot*8]

        # one-hot tiles: ohx[p, t*128 + d] = 1 iff dls[t*128+p] == d
        ohx = np.zeros((128, T_tot * 128), dtype=BF)
        i_all = np.arange(T_tot * 128)
        m = dls >= 0
        ohx[i_all[m] % 128, (i_all[m] // 128) * 128 + dls[m]] = 1

        # one-hot(x)^T: [32, nb*128] fp32
        xloc = np.zeros(nb * 128, dtype=np.int64)
        xloc[:npc] = x[c * npc:(c + 1) * npc]
        ohxt = np.zeros((M, nb * 128), dtype=np.float32)
        ohxt[xloc, np.arange(nb * 128)] = 1
        ohxt[:, npc:] = 0  # padded tail nodes: no contribution needed anyway

        cores.append({"idx": np.ascontiguousarray(idx_d),
                      "ohx": np.ascontiguousarray(ohx),
                      "ohxt": np.ascontiguousarray(ohxt)})
    sched = {"T": T, "off": off, "T_tot": T_tot,
             "bank_t0": bank_t0, "bank_t1": bank_t1}
    return cores, sched, lncnt


def permute_params(lambda_B0, lambda_Pi, lambda_Q, lambda_B):
    """Pure layout permutations (no compute): partition (g, c/k)-major views."""
    lamB0p = np.ascontiguousarray(
        np.transpose(np.asarray(lambda_B0, np.float32), (2, 0, 1)).reshape(G * C, M))
    lamPip = np.ascontiguousarray(np.asarray(lambda_Pi, np.float32).T)  # [G, C]
    lamQp = np.ascontiguousarray(
        np.transpose(np.asarray(lambda_Q, np.float32), (0, 3, 2, 1)).reshape(
            L - 1, G * C, C))
    lamBp = np.ascontiguousarray(
        np.transpose(np.asarray(lambda_B, np.float32), (0, 3, 1, 2)).reshape(
            L - 1, G * C, M))
    return {"lamB0p": lamB0p, "lamPip": lamPip, "lamQp": lamQp, "lamBp": lamBp}


def make_consts():
    ident_f = np.eye(128, dtype=np.float32)
    pp = np.arange(128) // 8
    maskg = (pp[:, None] == pp[None, :]).astype(np.float32)
    return {"ident_f": ident_f, "maskg": maskg}


# ---- bass kernel builder ----------------------------------------------------

def build_nc(cfg, sched):
    import concourse.bass as bass
    import concourse.bacc as bacc
    import concourse.mybir as mybir
    import concourse.tile as tile

    fp32 = mybir.dt.float32
    bf16 = mybir.dt.bfloat16
    i16 = mybir.dt.int16
    AX = mybir.AxisListType.X
    OP = mybir.AluOpType
    AF = mybir.ActivationFunctionType

    nb, npc, last_nn = cfg.nb, cfg.npc, cfg.last_nn
    T, off, T_tot = sched["T"], sched["off"], sched["T_tot"]
    bank_t0, bank_t1 = sched["bank_t0"], sched["bank_t1"]

    nc = bacc.Bacc("TRN2", target_bir_lowering=False, debug=False,
                   num_devices=cfg.ncores, num_swdge_queues=NQ)

    # ---- dram I/O
    idx_d = nc.dram_tensor("idx", [128, T_tot * 8], i16, kind="ExternalInput")
    ohx_d = nc.dram_tensor("ohx", [128, T_tot * 128], bf16, kind="ExternalInput")
    ohxt_d = nc.dram_tensor("ohxt", [M, nb * 128], fp32, kind="ExternalInput")
    lam_B0 = nc.dram_tensor("lamB0p", [128, M], fp32, kind="ExternalInput")
    lam_Pi = nc.dram_tensor("lamPip", [G, C], fp32, kind="ExternalInput")
    lam_Q = nc.dram_tensor("lamQp", [L - 1, 128, C], fp32, kind="ExternalInput")
    lam_B = nc.dram_tensor("lamBp", [L - 1, 128, M], fp32, kind="ExternalInput")
    pi_bounce = nc.dram_tensor("pi_bounce", [G * C], fp32)
    ident_f_d = nc.dram_tensor("ident_f", [128, 128], fp32, kind="ExternalInput")
    maskg_d = nc.dram_tensor("maskg", [128, 128], fp32, kind="ExternalInput")
    lls_d = nc.dram_tensor("lls", [npc, L * G], fp32, kind="ExternalOutput")

    h_slice = [[nc.dram_tensor(f"h_s{l}_{k}", [cfg.bank_nodes[k], CG], bf16)
                for k in range(NBANKS)] for l in range(L - 1)]
    h_full = [[nc.dram_tensor(f"h_f{l}_{k}", [cfg.ncores * cfg.bank_nodes[k], CG],
                              bf16, addr_space="Shared")
               for k in range(NBANKS)] for l in range(L - 1)]
    rgroups = [list(range(cfg.ncores))]

    # chunk table: list of (bank, t0, ntile); queue = index % NQ
    chunks = []
    tile2chunk = np.zeros(T_tot, dtype=np.int64)
    for k in range(NBANKS):
        for t0 in range(bank_t0[k], bank_t1[k], TG):
            ntile = min(TG, bank_t1[k] - t0)
            tile2chunk[t0:t0 + ntile] = len(chunks)
            chunks.append((k, t0, ntile))

    with tile.TileContext(nc) as tc:
        from contextlib import ExitStack
        with ExitStack() as ctx:
            res = ctx.enter_context(tc.tile_pool(name="res", bufs=1))
            sbp = ctx.enter_context(tc.tile_pool(name="sbp", bufs=3))
            gp = ctx.enter_context(tc.tile_pool(name="gp", bufs=3 * NQ))
            ohp = ctx.enter_context(tc.tile_pool(name="ohp", bufs=8))
            psp = ctx.enter_context(tc.tile_pool(name="psp", bufs=2, space="PSUM"))
            psa = ctx.enter_context(tc.tile_pool(name="psa", bufs=2, space="PSUM"))

            # ---- residents
            ident_f = res.tile([128, 128], fp32)
            nc.sync.dma_start(out=ident_f[:], in_=ident_f_d[:])
            maskg = res.tile([128, 128], fp32)
            nc.sync.dma_start(out=maskg[:], in_=maskg_d[:])
            idx_t = res.tile([128, T_tot * 8], i16)
            nc.sync.dma_start(out=idx_t[:], in_=idx_d[:])
            ohxt = res.tile([M, nb * 128], fp32)
            nc.sync.dma_start(out=ohxt[:], in_=ohxt_d[:])
            out_sb = res.tile([128, nb * 64], fp32)   # lls accumulator
            # per-block aggT accumulators (separate tiles -> independent deps)
            aggs = [res.tile([128, 128], fp32, name=f"aggb{_b}")
                    for _b in range(nb)]
            qbig = res.tile([128, 128], fp32)
            barrT = res.tile([32, 128], fp32)         # layer's B table [m, cg]
            pi_col = res.tile([128, 1], fp32)

            def softmax_free(raw, nfree, tag):
                mx = sbp.tile([raw.shape[0], 1], fp32, tag=f"{tag}mx")
                nc.vector.tensor_reduce(out=mx[:], in_=raw[:], axis=AX,
                                        op=OP.max, negate=True)
                ex = sbp.tile([raw.shape[0], nfree], fp32, tag=f"{tag}ex")
                nc.scalar.activation(out=ex[:], in_=raw[:], func=AF.Exp,
                                     bias=mx[:, 0:1], scale=1.0)
                sm = sbp.tile([raw.shape[0], 1], fp32, tag=f"{tag}sm")
                nc.vector.reduce_sum(out=sm[:], in_=ex[:], axis=AX)
                rs = sbp.tile([raw.shape[0], 1], fp32, tag=f"{tag}rs")
                nc.vector.reciprocal(out=rs[:], in_=sm[:])
                out = sbp.tile([raw.shape[0], nfree], fp32, tag=f"{tag}out")
                nc.vector.tensor_scalar(out=out[:], in0=ex[:], scalar1=rs[:, 0:1],
                                        scalar2=None, op0=OP.mult)
                return out

            def transpose_to(dest_sb, src_sb, pdim, fdim):
                ps = psp.tile([fdim, pdim], fp32, tag="trp", space="PSUM")
                nc.tensor.transpose(out=ps[:], in_=src_sb[:],
                                    identity=ident_f[:pdim, :pdim])
                nc.scalar.copy(out=dest_sb[:], in_=ps[:])

            def bank_of_block(b):
                return int(cfg.bank_of_block[b])

            max_bb = max(cfg.bank_blocks)
            hb_cur = {}  # bank -> SBUF tile accumulating this layer's h bank

            def emit_h(l, b, u, Z):
                """rz = 1/Z; h = u*rz (bf16) into bank SBUF tile; at bank end
                one batched DMA to h_slice + fire the AllGather."""
                kh = bank_of_block(b)
                bl = b - int(cfg.bank_first[kh])
                if bl == 0:
                    hb_cur[kh] = sbp.tile([128, max_bb * 128], bf16, tag="hb",
                                          name="hbank")
                hb = hb_cur[kh]
                rz = sbp.tile([128, G], fp32, tag="rz")
                nc.vector.reciprocal(out=rz[:], in_=Z[:])
                nc.vector.tensor_tensor(
                    out=hb[:, bl * 128:(bl + 1) * 128].rearrange(
                        "p (g c) -> p g c", c=C),
                    in0=u[:].rearrange("p (g c) -> p g c", c=C),
                    in1=rz[:].to_broadcast([128, G, C]), op=OP.mult)
                nblk = cfg.bank_blocks[kh]
                if bl == nblk - 1:
                    if b < nb - 1:  # no short block in this bank
                        nc.sync.dma_start(
                            out=h_slice[l][kh][:].rearrange(
                                "(b p) d -> p b d", p=128),
                            in_=hb[:, :nblk * 128].rearrange(
                                "p (b d) -> p b d", d=128))
                    else:
                        full = nblk - 1
                        if full > 0:
                            nc.sync.dma_start(
                                out=h_slice[l][kh][:full * 128, :].rearrange(
                                    "(b p) d -> p b d", p=128),
                                in_=hb[:, :full * 128].rearrange(
                                    "p (b d) -> p b d", d=128))
                        nc.sync.dma_start(
                            out=h_slice[l][kh][full * 128:, :],
                            in_=hb[:last_nn, full * 128:nblk * 128])
                    nc.gpsimd.collective_compute(
                        "AllGather", OP.bypass, replica_groups=rgroups,
                        ins=[h_slice[l][kh][:]], outs=[h_full[l][kh][:]])

            # ================= layer 0 =================
            braw = sbp.tile([128, M], fp32, tag="braw")
            nc.sync.dma_start(out=braw[:], in_=lam_B0[:])
            b0sm = softmax_free(braw, M, "b")
            praw = sbp.tile([G, C], fp32, tag="praw")
            nc.sync.dma_start(out=praw[:], in_=lam_Pi[:])
            pism = softmax_free(praw, C, "p")
            nc.sync.dma_start(out=pi_bounce[:].rearrange("(g c) -> g c", c=C),
                              in_=pism[:])
            nc.sync.dma_start(out=pi_col[:], in_=pi_bounce[:, None])
            b0p = sbp.tile([128, M], fp32, tag="b0p")
            nc.vector.tensor_scalar(out=b0p[:], in0=b0sm[:], scalar1=pi_col[:, 0:1],
                                    scalar2=None, op0=OP.mult)
            transpose_to(barrT, b0p, 128, 32)

            for b in range(nb):
                u0p = psp.tile([128, 128], fp32, tag="bx", space="PSUM")
                nc.tensor.matmul(out=u0p[:], lhsT=ohxt[:, b * 128:(b + 1) * 128],
                                 rhs=barrT[:], start=True, stop=True)
                u = sbp.tile([128, 128], fp32, tag="u")
                nc.scalar.copy(out=u[:], in_=u0p[:])
                Z = sbp.tile([128, G], fp32, tag="Z")
                nc.vector.reduce_sum(out=Z[:], in_=u[:].rearrange(
                    "p (g c) -> p g c", c=C), axis=AX)
                nc.scalar.activation(out=out_sb[:, b * 64:b * 64 + G], in_=Z[:],
                                     func=AF.Ln)
                emit_h(0, b, u, Z)

            # ================= graph layers =================
            for l in range(1, L):
                lq = l - 1

                qraw = sbp.tile([128, C], fp32, tag="qraw")
                nc.sync.dma_start(out=qraw[:], in_=lam_Q[lq])
                qsm = softmax_free(qraw, C, "q")  # [(g k), c]
                qsm_ap = qsm[:]
                qsm_bc = bass.AP(qsm_ap.tensor, qsm_ap.offset,
                                 [qsm_ap.ap[0], [0, G], qsm_ap.ap[1]])
                nc.vector.tensor_tensor(
                    out=qbig[:].rearrange("p (g c) -> p g c", c=C),
                    in0=qsm_bc,
                    in1=maskg[:].rearrange("p (g c) -> p g c", c=C),
                    op=OP.mult)
                braw2 = sbp.tile([128, M], fp32, tag="braw")
                nc.sync.dma_start(out=braw2[:], in_=lam_B[lq])
                bsm = softmax_free(braw2, M, "b")
                transpose_to(barrT, bsm, 128, 32)

                # ---- chunk caches (fresh per layer)
                gat_cache = {}
                oh_cache = {}

                def get_gat(gt, lq=lq):
                    ci = int(tile2chunk[gt])
                    if ci not in gat_cache:
                        k, t0, ntile = chunks[ci]
                        buf = gp.tile([128, ntile * 128], bf16, tag="g")
                        nc.gpsimd.dma_gather(
                            out_ap=buf[:].rearrange("p (t e) -> p t e", e=128),
                            in_ap=h_full[lq][k][:],
                            idxs_ap=idx_t[:, t0 * 8:(t0 + ntile) * 8],
                            num_idxs=ntile * 128,
                            num_idxs_reg=ntile * 128,
                            elem_size=128,
                            single_packet=False,
                            queue_num=ci % NQ)
                        gat_cache[ci] = buf
                    k, t0, ntile = chunks[ci]
                    return gat_cache[ci][:].rearrange(
                        "p (t e) -> p t e", e=128)[:, gt - t0, :]

                def get_oh(gt):
                    ci = int(tile2chunk[gt])
                    if ci not in oh_cache:
                        k, t0, ntile = chunks[ci]
                        buf = ohp.tile([128, ntile * 128], bf16, tag="oh")
                        nc.sync.dma_start(
                            out=buf[:], in_=ohx_d[:, t0 * 128:(t0 + ntile) * 128])
                        oh_cache[ci] = buf
                    k, t0, ntile = chunks[ci]
                    return oh_cache[ci][:, (gt - t0) * 128:(gt - t0 + 1) * 128]

                def finish(b, l=l):
                    sl = aggs[b][:]
                    qaT = psp.tile([128, 128], fp32, tag="qa", space="PSUM")
                    nc.tensor.matmul(out=qaT[:], lhsT=qbig[:], rhs=sl,
                                     start=True, stop=True)
                    qaTsb = sbp.tile([128, 128], fp32, tag="qaTsb")
                    nc.scalar.copy(out=qaTsb[:], in_=qaT[:])
                    qa2 = psp.tile([128, 128], fp32, tag="trp", space="PSUM")
                    nc.tensor.transpose(out=qa2[:], in_=qaTsb[:],
                                        identity=ident_f[:])
                    bx = psp.tile([128, 128], fp32, tag="bx", space="PSUM")
                    nc.tensor.matmul(out=bx[:],
                                     lhsT=ohxt[:, b * 128:(b + 1) * 128],
                                     rhs=barrT[:], start=True, stop=True)
                    bxsb = sbp.tile([128, 128], fp32, tag="bxsb")
                    nc.scalar.copy(out=bxsb[:], in_=bx[:])
                    u = sbp.tile([128, 128], fp32, tag="u")
                    nc.vector.tensor_tensor(out=u[:], in0=qa2[:], in1=bxsb[:],
                                            op=OP.mult)
                    Z = sbp.tile([128, G], fp32, tag="Z")
                    nc.vector.reduce_sum(out=Z[:], in_=u[:].rearrange(
                        "p (g c) -> p g c", c=C), axis=AX)
                    nc.scalar.activation(
                        out=out_sb[:, b * 64 + l * G:b * 64 + (l + 1) * G],
                        in_=Z[:], func=AF.Ln)
                    if l < L - 1:
                        emit_h(l, b, u, Z)

                pending_finish = None
                for k in range(NBANKS):
                    for b in range(nb):
                        nt = int(T[b, k])
                        ps = psa.tile([128, 128], fp32, tag="agg", space="PSUM")
                        for t in range(nt):
                            gt = int(off[k, b]) + t
                            nc.tensor.matmul(out=ps[:], lhsT=get_gat(gt),
                                             rhs=get_oh(gt),
                                             start=(t == 0), stop=(t == nt - 1))
                        sl = aggs[b][:]
                        if k == 0:
                            nc.scalar.copy(out=sl, in_=ps[:])
                        else:
                            nc.vector.tensor_tensor(out=sl, in0=ps[:], in1=sl,
                                                    op=OP.add)
                        if k == NBANKS - 1:
                            if pending_finish is not None:
                                finish(pending_finish)
                            pending_finish = b
                if pending_finish is not None:
                    finish(pending_finish)

            # ---- write lls out
            if nb > 1:
                nc.sync.dma_start(
                    out=lls_d[:(nb - 1) * 128, :].rearrange(
                        "(b p) c -> p b c", p=128),
                    in_=out_sb[:].rearrange("p (b c) -> p b c", c=64)[:, :nb - 1, :])
            nc.sync.dma_start(
                out=lls_d[(nb - 1) * 128:, :],
                in_=out_sb[:last_nn, (nb - 1) * 64:nb * 64])

    nc.compile()
    return nc


# ---- entry point ------------------------------------------------------------

def kernel(x, edge_index, lambda_B0, lambda_Pi, lambda_Q, lambda_B):
    cfg = Cfg()
    cores, sched, lncnt = preprocess(x, edge_index, cfg)
    consts = make_consts()
    nc = build_nc(cfg, sched)

    from concourse.bass_utils import run_bass_kernel_spmd
    params = permute_params(lambda_B0, lambda_Pi, lambda_Q, lambda_B)
    in_maps = []
    for c in range(cfg.ncores):
        m = dict(cores[c])
        m.update(params)
        m.update({k: np.ascontiguousarray(v) for k, v in consts.items()})
        in_maps.append(m)

    res = run_bass_kernel_spmd(nc, in_maps, core_ids=list(range(cfg.ncores)))
    out = np.concatenate([res.results[c]["lls"] for c in range(cfg.ncores)],
                         axis=0).reshape(N, L, G).astype(np.float32)
    out[:, 1:, :] -= lncnt[:, None, None]
    return out


# revision 23
# speedup vs baseline: 1.6582x; 1.0810x over previous
"""CGMM (Contextual Graph Markov Model) forward pass on 8 Trainium2 NeuronCores.

Self-contained: takes FULL inputs as numpy arrays, shards nodes/edges across
the 8 cores (graph parallel), runs a Bass/Tile kernel via
run_bass_kernel_spmd, returns the FULL [N, L, G] log-likelihood output.

Key layout (per core, nodes on partitions, cg = g*8 + c on free dim):
  layer 0:  u0[n, cg] = B0[c, x_n, g]*Pi[c, g]  via host-built one-hot(x) matmul
  layers 1..3:
            h split into NBANKS node-range banks; AllGather per bank overlaps
            the previous bank's compute (pipelined collectives)
            gather h_full_bank[src] per edge via dma_gather spread ROUND-ROBIN
            over 4 SWDGE queues (4 Q7 core pairs emit descriptors in parallel)
            aggT[cg, dst] = segment-sum via host-built one-hot matmuls
            (lhsT=gathered, rhs=onehot -> transposed aggregate, PSUM fp32),
            accumulated across banks in an SBUF fp32 tile
            QA^T = Qbig @ aggT; u = Bx * QA; Z = sum_c u; ll = log Z; h = u/Z
Host precomputes: edge sort/tiling, one-hot tiles (bf16), one-hot(x),
in-degree log-counts (applied as output post-processing: ll -= log cnt).
"""
import sys

sys.path.insert(0, "/opt/trn_rl_repo")

import numpy as np
import ml_dtypes

BF = ml_dtypes.bfloat16

# ---- problem sizes (hardcoded per contract) --------------------------------
N, E, C, M, G, L = 50000, 800000, 8, 32, 16, 4
NCORES = 8
CG = C * G  # 128
NBANKS = 2  # h banks (pipelined AllGather)
NQ = 4      # SWDGE queues used for dma_gather
TG = 16     # gather chunk size in 128-edge tiles


def split_blocks(nb, k):
    base = nb // k
    rem = nb % k
    return [base + (1 if i < rem else 0) for i in range(k)]


class Cfg:
    def __init__(self, n=N, e=E, ncores=NCORES):
        self.n = n
        self.e = e
        self.ncores = ncores
        self.npc = n // ncores
        self.nb = (self.npc + 127) // 128
        self.last_nn = self.npc - (self.nb - 1) * 128
        self.bank_blocks = split_blocks(self.nb, NBANKS)
        self.bank_first = np.concatenate([[0], np.cumsum(self.bank_blocks)])
        # nodes per bank within one core (last bank absorbs the short block)
        self.bank_node_start = [int(self.bank_first[k]) * 128
                                for k in range(NBANKS)]
        self.bank_nodes = [
            (int(self.bank_first[k + 1]) * 128 if k < NBANKS - 1 else self.npc)
            - self.bank_node_start[k]
            for k in range(NBANKS)]
        self.bank_of_block = np.searchsorted(
            self.bank_first[1:], np.arange(self.nb), side="right")


# ---- host preprocessing -----------------------------------------------------

def preprocess(x, edge_index, cfg):
    """Edge sort + tile schedule + one-hot tiles + degree counts (host)."""
    dst = np.asarray(edge_index[0], dtype=np.int64)
    src = np.asarray(edge_index[1], dtype=np.int64)
    x = np.asarray(x, dtype=np.int64)
    nc_, npc, nb = cfg.ncores, cfg.npc, cfg.nb

    owner = dst // npc
    per_core = []
    cnts = np.zeros((nc_, nb, NBANKS), dtype=np.int64)
    lncnt = np.zeros(cfg.n, dtype=np.float32)
    bob = cfg.bank_of_block
    bns = np.asarray(cfg.bank_node_start, dtype=np.int64)
    bsz = np.asarray(cfg.bank_nodes, dtype=np.int64)
    for c in range(nc_):
        sel = owner == c
        d = dst[sel] - c * npc
        s = src[sel]
        deg = np.bincount(d, minlength=npc)
        lncnt[c * npc:(c + 1) * npc] = np.log(np.maximum(deg, 1))
        b = d // 128
        dl = d % 128
        sown = s // npc
        soff = s % npc
        kb = bob[soff // 128]
        row = sown * bsz[kb] + (soff - bns[kb])
        per_core.append((b, dl, row, kb))
        key = b * NBANKS + kb
        cnts[c] = np.bincount(key, minlength=nb * NBANKS).reshape(nb, NBANKS)
    T = np.maximum(1, -(-cnts.max(axis=0) // 128))  # [nb, NBANKS]

    # tile offsets in (bank-major, block-minor) order
    off = np.zeros((NBANKS, nb), dtype=np.int64)
    pos = 0
    bank_t0 = []
    for k in range(NBANKS):
        bank_t0.append(pos)
        for b in range(nb):
            off[k, b] = pos
            pos += int(T[b, k])
    T_tot = pos
    bank_t1 = bank_t0[1:] + [T_tot]

    cores = []
    for c in range(nc_):
        b, dl, row, kb = per_core[c]
        idxs = np.zeros(T_tot * 128, dtype=np.int64)
        dls = np.full(T_tot * 128, -1, dtype=np.int64)
        order = np.argsort(kb * nb * 64 + b, kind="stable")
        b, dl, row, kb = b[order], dl[order], row[order], kb[order]
        # group boundaries: edges sorted by (bank, block)
        grp = kb * nb + b
        starts = np.searchsorted(grp, np.arange(NBANKS * nb), side="left")
        ends = np.searchsorted(grp, np.arange(NBANKS * nb), side="right")
        for k in range(NBANKS):
            for bb in range(nb):
                g0, g1 = starts[k * nb + bb], ends[k * nb + bb]
                if g1 <= g0:
                    continue
                a0 = off[k, bb] * 128
                idxs[a0:a0 + (g1 - g0)] = row[g0:g1]
                dls[a0:a0 + (g1 - g0)] = dl[g0:g1]

        # idx dram layout: [128, T_tot*8] int16; 16-row wrap, replicated 8x
        idx16 = idxs.astype(np.int16).reshape(-1, 16).T  # [16, T_tot*8]
        idx_d = np.tile(idx16, (8, 1))                   # [128, T_tot*8]

        # one-hot tiles: ohx[p, t*128 + d] = 1 iff dls[t*128+p] == d
        ohx = np.zeros((128, T_tot * 128), dtype=BF)
        i_all = np.arange(T_tot * 128)
        m = dls >= 0
        ohx[i_all[m] % 128, (i_all[m] // 128) * 128 + dls[m]] = 1

        # one-hot(x)^T: [32, nb*128] bf16
        xloc = np.zeros(nb * 128, dtype=np.int64)
        xloc[:npc] = x[c * npc:(c + 1) * npc]
        ohxt = np.zeros((M, nb * 128), dtype=BF)
        ohxt[xloc, np.arange(nb * 128)] = 1
        ohxt[:, npc:] = 0  # padded tail nodes: no contribution needed anyway

        cores.append({"idx": np.ascontiguousarray(idx_d),
                      "ohx": np.ascontiguousarray(ohx),
                      "ohxt": np.ascontiguousarray(ohxt)})
    sched = {"T": T, "off": off, "T_tot": T_tot,
             "bank_t0": bank_t0, "bank_t1": bank_t1}
    return cores, sched, lncnt


def permute_params(lambda_B0, lambda_Pi, lambda_Q, lambda_B):
    """Pure layout permutations (no compute): partition (g, c/k)-major views."""
    lamB0p = np.ascontiguousarray(
        np.transpose(np.asarray(lambda_B0, np.float32), (2, 0, 1)).reshape(G * C, M))
    lamPip = np.ascontiguousarray(np.asarray(lambda_Pi, np.float32).T)  # [G, C]
    lamQp = np.ascontiguousarray(
        np.transpose(np.asarray(lambda_Q, np.float32), (0, 3, 2, 1)).reshape(
            L - 1, G * C, C))
    lamBp = np.ascontiguousarray(
        np.transpose(np.asarray(lambda_B, np.float32), (0, 3, 1, 2)).reshape(
            L - 1, G * C, M))
    return {"lamB0p": lamB0p, "lamPip": lamPip, "lamQp": lamQp, "lamBp": lamBp}


def make_consts():
    ident_f = np.eye(128, dtype=np.float32)
    ident_b = np.eye(128, dtype=BF)
    pp = np.arange(128) // 8
    maskg = (pp[:, None] == pp[None, :]).astype(np.float32)
    return {"ident_f": ident_f, "ident_b": ident_b, "maskg": maskg}


# ---- bass kernel builder ----------------------------------------------------

def build_nc(cfg, sched):
    import concourse.bass as bass
    import concourse.bacc as bacc
    import concourse.mybir as mybir
    import concourse.tile as tile

    fp32 = mybir.dt.float32
    bf16 = mybir.dt.bfloat16
    i16 = mybir.dt.int16
    AX = mybir.AxisListType.X
    OP = mybir.AluOpType
    AF = mybir.ActivationFunctionType

    nb, npc, last_nn = cfg.nb, cfg.npc, cfg.last_nn
    T, off, T_tot = sched["T"], sched["off"], sched["T_tot"]
    bank_t0, bank_t1 = sched["bank_t0"], sched["bank_t1"]

    nc = bacc.Bacc("TRN2", target_bir_lowering=False, debug=False,
                   num_devices=cfg.ncores, num_swdge_queues=NQ)

    # ---- dram I/O
    idx_d = nc.dram_tensor("idx", [128, T_tot * 8], i16, kind="ExternalInput")
    ohx_d = nc.dram_tensor("ohx", [128, T_tot * 128], bf16, kind="ExternalInput")
    ohxt_d = nc.dram_tensor("ohxt", [M, nb * 128], bf16, kind="ExternalInput")
    lam_B0 = nc.dram_tensor("lamB0p", [128, M], fp32, kind="ExternalInput")
    lam_Pi = nc.dram_tensor("lamPip", [G, C], fp32, kind="ExternalInput")
    lam_Q = nc.dram_tensor("lamQp", [L - 1, 128, C], fp32, kind="ExternalInput")
    lam_B = nc.dram_tensor("lamBp", [L - 1, 128, M], fp32, kind="ExternalInput")
    pi_bounce = nc.dram_tensor("pi_bounce", [G * C], fp32)
    ident_f_d = nc.dram_tensor("ident_f", [128, 128], fp32, kind="ExternalInput")
    ident_b_d = nc.dram_tensor("ident_b", [128, 128], bf16, kind="ExternalInput")
    maskg_d = nc.dram_tensor("maskg", [128, 128], fp32, kind="ExternalInput")
    lls_d = nc.dram_tensor("lls", [npc, L * G], fp32, kind="ExternalOutput")

    h_slice = [[nc.dram_tensor(f"h_s{l}_{k}", [cfg.bank_nodes[k], CG], bf16)
                for k in range(NBANKS)] for l in range(L - 1)]
    h_full = [[nc.dram_tensor(f"h_f{l}_{k}", [cfg.ncores * cfg.bank_nodes[k], CG],
                              bf16, addr_space="Shared")
               for k in range(NBANKS)] for l in range(L - 1)]
    rgroups = [list(range(cfg.ncores))]

    # chunk table: list of (bank, t0, ntile); queue = index % NQ
    chunks = []
    tile2chunk = np.zeros(T_tot, dtype=np.int64)
    for k in range(NBANKS):
        for t0 in range(bank_t0[k], bank_t1[k], TG):
            ntile = min(TG, bank_t1[k] - t0)
            tile2chunk[t0:t0 + ntile] = len(chunks)
            chunks.append((k, t0, ntile))

    with tile.TileContext(nc) as tc:
        from contextlib import ExitStack
        with ExitStack() as ctx:
            res = ctx.enter_context(tc.tile_pool(name="res", bufs=1))
            sbp = ctx.enter_context(tc.tile_pool(name="sbp", bufs=3))
            gp = ctx.enter_context(tc.tile_pool(name="gp", bufs=3 * NQ))
            ohp = ctx.enter_context(tc.tile_pool(name="ohp", bufs=8))
            psp = ctx.enter_context(tc.tile_pool(name="psp", bufs=2, space="PSUM"))
            psa = ctx.enter_context(tc.tile_pool(name="psa", bufs=2, space="PSUM"))

            # ---- residents
            ident_f = res.tile([128, 128], fp32)
            nc.sync.dma_start(out=ident_f[:], in_=ident_f_d[:])
            ident_b = res.tile([128, 128], bf16)
            nc.sync.dma_start(out=ident_b[:], in_=ident_b_d[:])
            maskg = res.tile([128, 128], fp32)
            nc.sync.dma_start(out=maskg[:], in_=maskg_d[:])
            idx_t = res.tile([128, T_tot * 8], i16)
            nc.sync.dma_start(out=idx_t[:], in_=idx_d[:])
            ohxt = res.tile([M, nb * 128], bf16)
            nc.sync.dma_start(out=ohxt[:], in_=ohxt_d[:])
            out_sb = res.tile([128, nb * 64], fp32)   # lls accumulator
            # per-block bank-0 partial aggregates (bf16; re-injected into the
            # bank-1 PSUM chain via an identity matmul)
            aggb = [res.tile([128, 128], bf16, name=f"aggb{_b}")
                    for _b in range(nb)]
            qbig = res.tile([128, 128], bf16)
            barrT = res.tile([32, 128], bf16)         # layer's B table [m, cg]
            pi_col = res.tile([128, 1], fp32)

            def softmax_free(raw, nfree, tag):
                mx = sbp.tile([raw.shape[0], 1], fp32, tag=f"{tag}mx")
                nc.vector.tensor_reduce(out=mx[:], in_=raw[:], axis=AX,
                                        op=OP.max, negate=True)
                ex = sbp.tile([raw.shape[0], nfree], fp32, tag=f"{tag}ex")
                nc.scalar.activation(out=ex[:], in_=raw[:], func=AF.Exp,
                                     bias=mx[:, 0:1], scale=1.0)
                sm = sbp.tile([raw.shape[0], 1], fp32, tag=f"{tag}sm")
                nc.vector.reduce_sum(out=sm[:], in_=ex[:], axis=AX)
                rs = sbp.tile([raw.shape[0], 1], fp32, tag=f"{tag}rs")
                nc.vector.reciprocal(out=rs[:], in_=sm[:])
                out = sbp.tile([raw.shape[0], nfree], fp32, tag=f"{tag}out")
                nc.vector.tensor_scalar(out=out[:], in0=ex[:], scalar1=rs[:, 0:1],
                                        scalar2=None, op0=OP.mult)
                return out

            def transpose_to(dest_sb, src_sb, pdim, fdim):
                ps = psp.tile([fdim, pdim], fp32, tag="trp", space="PSUM")
                nc.tensor.transpose(out=ps[:], in_=src_sb[:],
                                    identity=ident_f[:pdim, :pdim])
                nc.scalar.copy(out=dest_sb[:], in_=ps[:])

            def bank_of_block(b):
                return int(cfg.bank_of_block[b])

            max_bb = max(cfg.bank_blocks)
            hb_cur = {}  # bank -> SBUF tile accumulating this layer's h bank

            def emit_h(l, b, u_ap, Z):
                """rz = 1/Z; h = u*rz (bf16) into bank SBUF tile; at bank end
                one batched DMA to h_slice + fire the AllGather. u_ap: AP."""
                kh = bank_of_block(b)
                bl = b - int(cfg.bank_first[kh])
                if bl == 0:
                    hb_cur[kh] = sbp.tile([128, max_bb * 128], bf16, tag="hb",
                                          name="hbank")
                hb = hb_cur[kh]
                rz = sbp.tile([128, G], fp32, tag="rz")
                nc.vector.reciprocal(out=rz[:], in_=Z[:])
                nc.vector.tensor_tensor(
                    out=hb[:, bl * 128:(bl + 1) * 128].rearrange(
                        "p (g c) -> p g c", c=C),
                    in0=u_ap.rearrange("p (g c) -> p g c", c=C),
                    in1=rz[:].to_broadcast([128, G, C]), op=OP.mult)
                nblk = cfg.bank_blocks[kh]
                if bl == nblk - 1:
                    if b < nb - 1:  # no short block in this bank
                        nc.sync.dma_start(
                            out=h_slice[l][kh][:].rearrange(
                                "(b p) d -> p b d", p=128),
                            in_=hb[:, :nblk * 128].rearrange(
                                "p (b d) -> p b d", d=128))
                    else:
                        full = nblk - 1
                        if full > 0:
                            nc.sync.dma_start(
                                out=h_slice[l][kh][:full * 128, :].rearrange(
                                    "(b p) d -> p b d", p=128),
                                in_=hb[:, :full * 128].rearrange(
                                    "p (b d) -> p b d", d=128))
                        nc.sync.dma_start(
                            out=h_slice[l][kh][full * 128:, :],
                            in_=hb[:last_nn, full * 128:nblk * 128])
                    nc.gpsimd.collective_compute(
                        "AllGather", OP.bypass, replica_groups=rgroups,
                        ins=[h_slice[l][kh][:]], outs=[h_full[l][kh][:]])

            # ================= layer 0 =================
            braw = sbp.tile([128, M], fp32, tag="braw")
            nc.sync.dma_start(out=braw[:], in_=lam_B0[:])
            b0sm = softmax_free(braw, M, "b")
            praw = sbp.tile([G, C], fp32, tag="praw")
            nc.sync.dma_start(out=praw[:], in_=lam_Pi[:])
            pism = softmax_free(praw, C, "p")
            nc.sync.dma_start(out=pi_bounce[:].rearrange("(g c) -> g c", c=C),
                              in_=pism[:])
            nc.sync.dma_start(out=pi_col[:], in_=pi_bounce[:, None])
            b0p = sbp.tile([128, M], fp32, tag="b0p")
            nc.vector.tensor_scalar(out=b0p[:], in0=b0sm[:], scalar1=pi_col[:, 0:1],
                                    scalar2=None, op0=OP.mult)
            transpose_to(barrT, b0p, 128, 32)

            l0_ps = {}
            for i in range(nb + 1):
                if i < nb:
                    u0p = psp.tile([128, 128], fp32, tag="bx", space="PSUM")
                    nc.tensor.matmul(out=u0p[:],
                                     lhsT=ohxt[:, i * 128:(i + 1) * 128],
                                     rhs=barrT[:], start=True, stop=True)
                    l0_ps[i] = u0p
                if i >= 1:
                    b = i - 1
                    ps = l0_ps.pop(b)
                    Z = sbp.tile([128, G], fp32, tag="Z")
                    nc.vector.reduce_sum(out=Z[:], in_=ps[:].rearrange(
                        "p (g c) -> p g c", c=C), axis=AX)
                    nc.scalar.activation(out=out_sb[:, b * 64:b * 64 + G],
                                         in_=Z[:], func=AF.Ln)
                    emit_h(0, b, ps[:], Z)

            # ================= graph layers =================
            for l in range(1, L):
                lq = l - 1

                qraw = sbp.tile([128, C], fp32, tag="qraw")
                nc.sync.dma_start(out=qraw[:], in_=lam_Q[lq])
                qsm = softmax_free(qraw, C, "q")  # [(g k), c]
                qsm_ap = qsm[:]
                qsm_bc = bass.AP(qsm_ap.tensor, qsm_ap.offset,
                                 [qsm_ap.ap[0], [0, G], qsm_ap.ap[1]])
                nc.vector.tensor_tensor(
                    out=qbig[:].rearrange("p (g c) -> p g c", c=C),
                    in0=qsm_bc,
                    in1=maskg[:].rearrange("p (g c) -> p g c", c=C),
                    op=OP.mult)
                braw2 = sbp.tile([128, M], fp32, tag="braw")
                nc.sync.dma_start(out=braw2[:], in_=lam_B[lq])
                bsm = softmax_free(braw2, M, "b")
                transpose_to(barrT, bsm, 128, 32)

                # ---- chunk caches (fresh per layer)
                gat_cache = {}
                oh_cache = {}

                def get_gat(gt, lq=lq):
                    ci = int(tile2chunk[gt])
                    if ci not in gat_cache:
                        k, t0, ntile = chunks[ci]
                        buf = gp.tile([128, ntile * 128], bf16, tag="g")
                        nc.gpsimd.dma_gather(
                            out_ap=buf[:].rearrange("p (t e) -> p t e", e=128),
                            in_ap=h_full[lq][k][:],
                            idxs_ap=idx_t[:, t0 * 8:(t0 + ntile) * 8],
                            num_idxs=ntile * 128,
                            num_idxs_reg=ntile * 128,
                            elem_size=128,
                            single_packet=False,
                            queue_num=ci % NQ)
                        gat_cache[ci] = buf
                    k, t0, ntile = chunks[ci]
                    return gat_cache[ci][:].rearrange(
                        "p (t e) -> p t e", e=128)[:, gt - t0, :]

                def get_oh(gt):
                    ci = int(tile2chunk[gt])
                    if ci not in oh_cache:
                        k, t0, ntile = chunks[ci]
                        buf = ohp.tile([128, ntile * 128], bf16, tag="oh")
                        nc.sync.dma_start(
                            out=buf[:], in_=ohx_d[:, t0 * 128:(t0 + ntile) * 128])
                        oh_cache[ci] = buf
                    k, t0, ntile = chunks[ci]
                    return oh_cache[ci][:, (gt - t0) * 128:(gt - t0 + 1) * 128]

                # ---- phase A: bank-0 sweep -> bf16 partial aggregates
                for b in range(nb):
                    nt = int(T[b, 0])
                    ps = psa.tile([128, 128], fp32, tag="agg", space="PSUM")
                    for t in range(nt):
                        gt = int(off[0, b]) + t
                        nc.tensor.matmul(out=ps[:], lhsT=get_gat(gt),
                                         rhs=get_oh(gt),
                                         start=(t == 0), stop=(t == nt - 1))
                    nc.scalar.copy(out=aggb[b][:], in_=ps[:])

                # ---- phase B: bank-1 block-major, 3-stage pipelined finish
                aggF_d = {}
                s1_d = {}
                for i in range(nb + 2):
                    if i < nb:
                        b = i
                        nt = int(T[b, 1])
                        ps = psa.tile([128, 128], fp32, tag="agg", space="PSUM")
                        # re-inject bank-0 partial: I^T @ aggb = aggb
                        nc.tensor.matmul(out=ps[:], lhsT=ident_b[:],
                                         rhs=aggb[b][:], start=True, stop=False)
                        for t in range(nt):
                            gt = int(off[1, b]) + t
                            nc.tensor.matmul(out=ps[:], lhsT=get_gat(gt),
                                             rhs=get_oh(gt),
                                             start=False, stop=(t == nt - 1))
                        aggF = sbp.tile([128, 128], bf16, tag="aggF")
                        nc.scalar.copy(out=aggF[:], in_=ps[:])
                        aggF_d[b] = aggF
                    if 1 <= i <= nb:
                        b = i - 1
                        qaT = psp.tile([128, 128], fp32, tag="qa", space="PSUM")
                        nc.tensor.matmul(out=qaT[:], lhsT=qbig[:],
                                         rhs=aggF_d.pop(b)[:],
                                         start=True, stop=True)
                        qaTsb = sbp.tile([128, 128], fp32, tag="qaTsb")
                        nc.scalar.copy(out=qaTsb[:], in_=qaT[:])
                        bx = psp.tile([128, 128], fp32, tag="bx", space="PSUM")
                        nc.tensor.matmul(out=bx[:],
                                         lhsT=ohxt[:, b * 128:(b + 1) * 128],
                                         rhs=barrT[:], start=True, stop=True)
                        bxsb = sbp.tile([128, 128], fp32, tag="bxsb")
                        nc.scalar.copy(out=bxsb[:], in_=bx[:])
                        s1_d[b] = (qaTsb, bxsb)
                    if 2 <= i:
                        b = i - 2
                        qaTsb, bxsb = s1_d.pop(b)
                        qa2 = psp.tile([128, 128], fp32, tag="trp", space="PSUM")
                        nc.tensor.transpose(out=qa2[:], in_=qaTsb[:],
                                            identity=ident_f[:])
                        u = sbp.tile([128, 128], fp32, tag="u")
                        nc.vector.tensor_tensor(out=u[:], in0=qa2[:], in1=bxsb[:],
                                                op=OP.mult)
                        Z = sbp.tile([128, G], fp32, tag="Z")
                        nc.vector.reduce_sum(out=Z[:], in_=u[:].rearrange(
                            "p (g c) -> p g c", c=C), axis=AX)
                        nc.scalar.activation(
                            out=out_sb[:, b * 64 + l * G:b * 64 + (l + 1) * G],
                            in_=Z[:], func=AF.Ln)
                        if l < L - 1:
                            emit_h(l, b, u[:], Z)

            # ---- write lls out
            if nb > 1:
                nc.sync.dma_start(
                    out=lls_d[:(nb - 1) * 128, :].rearrange(
                        "(b p) c -> p b c", p=128),
                    in_=out_sb[:].rearrange("p (b c) -> p b c", c=64)[:, :nb - 1, :])
            nc.sync.dma_start(
                out=lls_d[(nb - 1) * 128:, :],
                in_=out_sb[:last_nn, (nb - 1) * 64:nb * 64])

    nc.compile()
    return nc


# ---- entry point ------------------------------------------------------------

def kernel(x, edge_index, lambda_B0, lambda_Pi, lambda_Q, lambda_B):
    cfg = Cfg()
    cores, sched, lncnt = preprocess(x, edge_index, cfg)
    consts = make_consts()
    nc = build_nc(cfg, sched)

    from concourse.bass_utils import run_bass_kernel_spmd
    params = permute_params(lambda_B0, lambda_Pi, lambda_Q, lambda_B)
    in_maps = []
    for c in range(cfg.ncores):
        m = dict(cores[c])
        m.update(params)
        m.update({k: np.ascontiguousarray(v) for k, v in consts.items()})
        in_maps.append(m)

    res = run_bass_kernel_spmd(nc, in_maps, core_ids=list(range(cfg.ncores)))
    out = np.concatenate([res.results[c]["lls"] for c in range(cfg.ncores)],
                         axis=0).reshape(N, L, G).astype(np.float32)
    out[:, 1:, :] -= lncnt[:, None, None]
    return out


# revision 25
# speedup vs baseline: 1.6734x; 1.0092x over previous
"""CGMM (Contextual Graph Markov Model) forward pass on 8 Trainium2 NeuronCores.

Self-contained: takes FULL inputs as numpy arrays, shards nodes/edges across
the 8 cores (graph parallel), runs a Bass/Tile kernel via
run_bass_kernel_spmd, returns the FULL [N, L, G] log-likelihood output.

Key layout (per core, nodes on partitions, cg = g*8 + c on free dim):
  layer 0:  u0[n, cg] = B0[c, x_n, g]*Pi[c, g]  via host-built one-hot(x) matmul
  layers 1..3:
            h split into NBANKS node-range banks; AllGather per bank overlaps
            the previous bank's compute (pipelined collectives)
            gather h_full_bank[src] per edge via dma_gather spread ROUND-ROBIN
            over 4 SWDGE queues (4 Q7 core pairs emit descriptors in parallel)
            aggT[cg, dst] = segment-sum via host-built one-hot matmuls
            (lhsT=gathered, rhs=onehot -> transposed aggregate, PSUM fp32),
            accumulated across banks in an SBUF fp32 tile
            QA^T = Qbig @ aggT; u = Bx * QA; Z = sum_c u; ll = log Z; h = u/Z
Host precomputes: edge sort/tiling, one-hot tiles (bf16), one-hot(x),
in-degree log-counts (applied as output post-processing: ll -= log cnt).
"""
import sys

sys.path.insert(0, "/opt/trn_rl_repo")

import numpy as np
import ml_dtypes

BF = ml_dtypes.bfloat16

# ---- problem sizes (hardcoded per contract) --------------------------------
N, E, C, M, G, L = 50000, 800000, 8, 32, 16, 4
NCORES = 8
CG = C * G  # 128
NBANKS = 2  # h banks (pipelined AllGather)
NQ = 4      # SWDGE queues used for dma_gather
TG = 16     # gather chunk size in 128-edge tiles


def split_blocks(nb, k):
    base = nb // k
    rem = nb % k
    return [base + (1 if i < rem else 0) for i in range(k)]


class Cfg:
    def __init__(self, n=N, e=E, ncores=NCORES):
        self.n = n
        self.e = e
        self.ncores = ncores
        self.npc = n // ncores
        self.nb = (self.npc + 127) // 128
        self.last_nn = self.npc - (self.nb - 1) * 128
        self.bank_blocks = split_blocks(self.nb, NBANKS)
        self.bank_first = np.concatenate([[0], np.cumsum(self.bank_blocks)])
        # nodes per bank within one core (last bank absorbs the short block)
        self.bank_node_start = [int(self.bank_first[k]) * 128
                                for k in range(NBANKS)]
        self.bank_nodes = [
            (int(self.bank_first[k + 1]) * 128 if k < NBANKS - 1 else self.npc)
            - self.bank_node_start[k]
            for k in range(NBANKS)]
        self.bank_of_block = np.searchsorted(
            self.bank_first[1:], np.arange(self.nb), side="right")


# ---- host preprocessing -----------------------------------------------------

def preprocess(x, edge_index, cfg):
    """Edge sort + tile schedule + one-hot tiles + degree counts (host)."""
    dst = np.asarray(edge_index[0], dtype=np.int64)
    src = np.asarray(edge_index[1], dtype=np.int64)
    x = np.asarray(x, dtype=np.int64)
    nc_, npc, nb = cfg.ncores, cfg.npc, cfg.nb

    owner = dst // npc
    per_core = []
    cnts = np.zeros((nc_, nb, NBANKS), dtype=np.int64)
    lncnt = np.zeros(cfg.n, dtype=np.float32)
    bob = cfg.bank_of_block
    bns = np.asarray(cfg.bank_node_start, dtype=np.int64)
    bsz = np.asarray(cfg.bank_nodes, dtype=np.int64)
    for c in range(nc_):
        sel = owner == c
        d = dst[sel] - c * npc
        s = src[sel]
        deg = np.bincount(d, minlength=npc)
        lncnt[c * npc:(c + 1) * npc] = np.log(np.maximum(deg, 1))
        b = d // 128
        dl = d % 128
        sown = s // npc
        soff = s % npc
        kb = bob[soff // 128]
        row = sown * bsz[kb] + (soff - bns[kb])
        per_core.append((b, dl, row, kb))
        key = b * NBANKS + kb
        cnts[c] = np.bincount(key, minlength=nb * NBANKS).reshape(nb, NBANKS)
    T = np.maximum(1, -(-cnts.max(axis=0) // 128))  # [nb, NBANKS]

    # tile offsets in (bank-major, block-minor) order
    off = np.zeros((NBANKS, nb), dtype=np.int64)
    pos = 0
    bank_t0 = []
    for k in range(NBANKS):
        bank_t0.append(pos)
        for b in range(nb):
            off[k, b] = pos
            pos += int(T[b, k])
    T_tot = pos
    bank_t1 = bank_t0[1:] + [T_tot]

    cores = []
    for c in range(nc_):
        b, dl, row, kb = per_core[c]
        idxs = np.zeros(T_tot * 128, dtype=np.int64)
        dls = np.full(T_tot * 128, -1, dtype=np.int64)
        order = np.argsort(kb * nb * 64 + b, kind="stable")
        b, dl, row, kb = b[order], dl[order], row[order], kb[order]
        # group boundaries: edges sorted by (bank, block)
        grp = kb * nb + b
        starts = np.searchsorted(grp, np.arange(NBANKS * nb), side="left")
        ends = np.searchsorted(grp, np.arange(NBANKS * nb), side="right")
        for k in range(NBANKS):
            for bb in range(nb):
                g0, g1 = starts[k * nb + bb], ends[k * nb + bb]
                if g1 <= g0:
                    continue
                a0 = off[k, bb] * 128
                idxs[a0:a0 + (g1 - g0)] = row[g0:g1]
                dls[a0:a0 + (g1 - g0)] = dl[g0:g1]

        # idx dram layout: [128, T_tot*8] int16; 16-row wrap, replicated 8x
        idx16 = idxs.astype(np.int16).reshape(-1, 16).T  # [16, T_tot*8]
        idx_d = np.tile(idx16, (8, 1))                   # [128, T_tot*8]

        # one-hot tiles: ohx[p, t*128 + d] = 1 iff dls[t*128+p] == d
        ohx = np.zeros((128, T_tot * 128), dtype=BF)
        i_all = np.arange(T_tot * 128)
        m = dls >= 0
        ohx[i_all[m] % 128, (i_all[m] // 128) * 128 + dls[m]] = 1

        # one-hot(x)^T: [32, nb*128] bf16
        xloc = np.zeros(nb * 128, dtype=np.int64)
        xloc[:npc] = x[c * npc:(c + 1) * npc]
        ohxt = np.zeros((M, nb * 128), dtype=BF)
        ohxt[xloc, np.arange(nb * 128)] = 1
        ohxt[:, npc:] = 0  # padded tail nodes: no contribution needed anyway

        cores.append({"idx": np.ascontiguousarray(idx_d),
                      "ohx": np.ascontiguousarray(ohx),
                      "ohxt": np.ascontiguousarray(ohxt)})
    sched = {"T": T, "off": off, "T_tot": T_tot,
             "bank_t0": bank_t0, "bank_t1": bank_t1}
    return cores, sched, lncnt


def permute_params(lambda_B0, lambda_Pi, lambda_Q, lambda_B):
    """Pure layout permutations (no compute): partition (g, c/k)-major views."""
    lamB0p = np.ascontiguousarray(
        np.transpose(np.asarray(lambda_B0, np.float32), (2, 0, 1)).reshape(G * C, M))
    lamPip = np.ascontiguousarray(np.asarray(lambda_Pi, np.float32).T)  # [G, C]
    lamQp = np.ascontiguousarray(
        np.transpose(np.asarray(lambda_Q, np.float32), (0, 3, 2, 1)).reshape(
            L - 1, G * C, C))
    lamBp = np.ascontiguousarray(
        np.transpose(np.asarray(lambda_B, np.float32), (0, 3, 1, 2)).reshape(
            L - 1, G * C, M))
    return {"lamB0p": lamB0p, "lamPip": lamPip, "lamQp": lamQp, "lamBp": lamBp}


def make_consts():
    ident_f = np.eye(128, dtype=np.float32)
    ident_b = np.eye(128, dtype=BF)
    pp = np.arange(128) // 8
    maskg = (pp[:, None] == pp[None, :]).astype(np.float32)
    return {"ident_f": ident_f, "ident_b": ident_b, "maskg": maskg}


# ---- bass kernel builder ----------------------------------------------------

def build_nc(cfg, sched):
    import concourse.bass as bass
    import concourse.bacc as bacc
    import concourse.mybir as mybir
    import concourse.tile as tile

    fp32 = mybir.dt.float32
    bf16 = mybir.dt.bfloat16
    i16 = mybir.dt.int16
    AX = mybir.AxisListType.X
    OP = mybir.AluOpType
    AF = mybir.ActivationFunctionType

    nb, npc, last_nn = cfg.nb, cfg.npc, cfg.last_nn
    T, off, T_tot = sched["T"], sched["off"], sched["T_tot"]
    bank_t0, bank_t1 = sched["bank_t0"], sched["bank_t1"]

    nc = bacc.Bacc("TRN2", target_bir_lowering=False, debug=False,
                   num_devices=cfg.ncores, num_swdge_queues=NQ)

    # ---- dram I/O
    idx_d = nc.dram_tensor("idx", [128, T_tot * 8], i16, kind="ExternalInput")
    ohx_d = nc.dram_tensor("ohx", [128, T_tot * 128], bf16, kind="ExternalInput")
    ohxt_d = nc.dram_tensor("ohxt", [M, nb * 128], bf16, kind="ExternalInput")
    lam_B0 = nc.dram_tensor("lamB0p", [128, M], fp32, kind="ExternalInput")
    lam_Pi = nc.dram_tensor("lamPip", [G, C], fp32, kind="ExternalInput")
    lam_Q = nc.dram_tensor("lamQp", [L - 1, 128, C], fp32, kind="ExternalInput")
    lam_B = nc.dram_tensor("lamBp", [L - 1, 128, M], fp32, kind="ExternalInput")
    pi_bounce = nc.dram_tensor("pi_bounce", [G * C], fp32)
    ident_f_d = nc.dram_tensor("ident_f", [128, 128], fp32, kind="ExternalInput")
    ident_b_d = nc.dram_tensor("ident_b", [128, 128], bf16, kind="ExternalInput")
    maskg_d = nc.dram_tensor("maskg", [128, 128], fp32, kind="ExternalInput")
    lls_d = nc.dram_tensor("lls", [npc, L * G], fp32, kind="ExternalOutput")

    h_slice = [[nc.dram_tensor(f"h_s{l}_{k}", [cfg.bank_nodes[k], CG], bf16)
                for k in range(NBANKS)] for l in range(L - 1)]
    h_full = [[nc.dram_tensor(f"h_f{l}_{k}", [cfg.ncores * cfg.bank_nodes[k], CG],
                              bf16, addr_space="Shared")
               for k in range(NBANKS)] for l in range(L - 1)]
    rgroups = [list(range(cfg.ncores))]

    # chunk table: list of (bank, t0, ntile); queue = index % NQ
    chunks = []
    tile2chunk = np.zeros(T_tot, dtype=np.int64)
    for k in range(NBANKS):
        for t0 in range(bank_t0[k], bank_t1[k], TG):
            ntile = min(TG, bank_t1[k] - t0)
            tile2chunk[t0:t0 + ntile] = len(chunks)
            chunks.append((k, t0, ntile))

    with tile.TileContext(nc) as tc:
        from contextlib import ExitStack
        with ExitStack() as ctx:
            res = ctx.enter_context(tc.tile_pool(name="res", bufs=1))
            sbp = ctx.enter_context(tc.tile_pool(name="sbp", bufs=3))
            gp = ctx.enter_context(tc.tile_pool(name="gp", bufs=3 * NQ))
            ohp = ctx.enter_context(tc.tile_pool(name="ohp", bufs=8))
            psp = ctx.enter_context(tc.tile_pool(name="psp", bufs=2, space="PSUM"))
            psa = ctx.enter_context(tc.tile_pool(name="psa", bufs=2, space="PSUM"))

            # ---- residents
            ident_f = res.tile([128, 128], fp32)
            nc.sync.dma_start(out=ident_f[:], in_=ident_f_d[:])
            ident_b = res.tile([128, 128], bf16)
            nc.sync.dma_start(out=ident_b[:], in_=ident_b_d[:])
            maskg = res.tile([128, 128], fp32)
            nc.sync.dma_start(out=maskg[:], in_=maskg_d[:])
            idx_t = res.tile([128, T_tot * 8], i16)
            nc.sync.dma_start(out=idx_t[:], in_=idx_d[:])
            ohxt = res.tile([M, nb * 128], bf16)
            nc.sync.dma_start(out=ohxt[:], in_=ohxt_d[:])
            out_sb = res.tile([128, nb * 64], fp32)   # lls accumulator
            # per-block bank-0 partial aggregates (bf16; re-injected into the
            # bank-1 PSUM chain via an identity matmul)
            aggb = [res.tile([128, 128], bf16, name=f"aggb{_b}")
                    for _b in range(nb)]
            qbig = res.tile([128, 128], bf16)
            barrT = res.tile([32, 128], bf16)         # layer's B table [m, cg]
            pi_col = res.tile([128, 1], fp32)

            def softmax_free(raw, nfree, tag):
                mx = sbp.tile([raw.shape[0], 1], fp32, tag=f"{tag}mx")
                nc.vector.tensor_reduce(out=mx[:], in_=raw[:], axis=AX,
                                        op=OP.max, negate=True)
                ex = sbp.tile([raw.shape[0], nfree], fp32, tag=f"{tag}ex")
                nc.scalar.activation(out=ex[:], in_=raw[:], func=AF.Exp,
                                     bias=mx[:, 0:1], scale=1.0)
                sm = sbp.tile([raw.shape[0], 1], fp32, tag=f"{tag}sm")
                nc.vector.reduce_sum(out=sm[:], in_=ex[:], axis=AX)
                rs = sbp.tile([raw.shape[0], 1], fp32, tag=f"{tag}rs")
                nc.vector.reciprocal(out=rs[:], in_=sm[:])
                out = sbp.tile([raw.shape[0], nfree], fp32, tag=f"{tag}out")
                nc.vector.tensor_scalar(out=out[:], in0=ex[:], scalar1=rs[:, 0:1],
                                        scalar2=None, op0=OP.mult)
                return out

            def transpose_to(dest_sb, src_sb, pdim, fdim):
                ps = psp.tile([fdim, pdim], fp32, tag="trp", space="PSUM")
                nc.tensor.transpose(out=ps[:], in_=src_sb[:],
                                    identity=ident_f[:pdim, :pdim])
                nc.scalar.copy(out=dest_sb[:], in_=ps[:])

            def bank_of_block(b):
                return int(cfg.bank_of_block[b])

            max_bb = max(cfg.bank_blocks)
            hb_cur = {}  # bank -> SBUF tile accumulating this layer's h bank

            def emit_h(l, b, u_ap, Z):
                """rz = 1/Z; h = u*rz (bf16) into bank SBUF tile; at bank end
                one batched DMA to h_slice + fire the AllGather. u_ap: AP."""
                kh = bank_of_block(b)
                bl = b - int(cfg.bank_first[kh])
                if bl == 0:
                    hb_cur[kh] = sbp.tile([128, max_bb * 128], bf16, tag="hb",
                                          name="hbank")
                hb = hb_cur[kh]
                rz = sbp.tile([128, G], fp32, tag="rz")
                nc.vector.reciprocal(out=rz[:], in_=Z[:])
                nc.vector.tensor_tensor(
                    out=hb[:, bl * 128:(bl + 1) * 128].rearrange(
                        "p (g c) -> p g c", c=C),
                    in0=u_ap.rearrange("p (g c) -> p g c", c=C),
                    in1=rz[:].to_broadcast([128, G, C]), op=OP.mult)
                nblk = cfg.bank_blocks[kh]
                if bl == nblk - 1:
                    # ACT-engine HWDGE: keeps the Sync FIFO free for oh loads
                    if b < nb - 1:  # no short block in this bank
                        nc.scalar.dma_start(
                            out=h_slice[l][kh][:].rearrange(
                                "(b p) d -> p b d", p=128),
                            in_=hb[:, :nblk * 128].rearrange(
                                "p (b d) -> p b d", d=128))
                    else:
                        full = nblk - 1
                        if full > 0:
                            nc.scalar.dma_start(
                                out=h_slice[l][kh][:full * 128, :].rearrange(
                                    "(b p) d -> p b d", p=128),
                                in_=hb[:, :full * 128].rearrange(
                                    "p (b d) -> p b d", d=128))
                        nc.scalar.dma_start(
                            out=h_slice[l][kh][full * 128:, :],
                            in_=hb[:last_nn, full * 128:nblk * 128])
                    nc.gpsimd.collective_compute(
                        "AllGather", OP.bypass, replica_groups=rgroups,
                        ins=[h_slice[l][kh][:]], outs=[h_full[l][kh][:]])

            # ================= layer 0 =================
            braw = sbp.tile([128, M], fp32, tag="braw")
            nc.sync.dma_start(out=braw[:], in_=lam_B0[:])
            b0sm = softmax_free(braw, M, "b")
            praw = sbp.tile([G, C], fp32, tag="praw")
            nc.sync.dma_start(out=praw[:], in_=lam_Pi[:])
            pism = softmax_free(praw, C, "p")
            nc.sync.dma_start(out=pi_bounce[:].rearrange("(g c) -> g c", c=C),
                              in_=pism[:])
            nc.sync.dma_start(out=pi_col[:], in_=pi_bounce[:, None])
            b0p = sbp.tile([128, M], fp32, tag="b0p")
            nc.vector.tensor_scalar(out=b0p[:], in0=b0sm[:], scalar1=pi_col[:, 0:1],
                                    scalar2=None, op0=OP.mult)
            transpose_to(barrT, b0p, 128, 32)

            l0_ps = {}
            for i in range(nb + 1):
                if i < nb:
                    u0p = psp.tile([128, 128], fp32,
                                   tag="bx" if i % 2 else "qa", space="PSUM")
                    nc.tensor.matmul(out=u0p[:],
                                     lhsT=ohxt[:, i * 128:(i + 1) * 128],
                                     rhs=barrT[:], start=True, stop=True)
                    l0_ps[i] = u0p
                if i >= 1:
                    b = i - 1
                    ps = l0_ps.pop(b)
                    Z = sbp.tile([128, G], fp32, tag="Z")
                    nc.vector.reduce_sum(out=Z[:], in_=ps[:].rearrange(
                        "p (g c) -> p g c", c=C), axis=AX)
                    nc.scalar.activation(out=out_sb[:, b * 64:b * 64 + G],
                                         in_=Z[:], func=AF.Ln)
                    emit_h(0, b, ps[:], Z)

            # ================= graph layers =================
            for l in range(1, L):
                lq = l - 1

                qraw = sbp.tile([128, C], fp32, tag="qraw")
                nc.sync.dma_start(out=qraw[:], in_=lam_Q[lq])
                qsm = softmax_free(qraw, C, "q")  # [(g k), c]
                qsm_ap = qsm[:]
                qsm_bc = bass.AP(qsm_ap.tensor, qsm_ap.offset,
                                 [qsm_ap.ap[0], [0, G], qsm_ap.ap[1]])
                nc.vector.tensor_tensor(
                    out=qbig[:].rearrange("p (g c) -> p g c", c=C),
                    in0=qsm_bc,
                    in1=maskg[:].rearrange("p (g c) -> p g c", c=C),
                    op=OP.mult)
                braw2 = sbp.tile([128, M], fp32, tag="braw")
                nc.sync.dma_start(out=braw2[:], in_=lam_B[lq])
                bsm = softmax_free(braw2, M, "b")
                transpose_to(barrT, bsm, 128, 32)

                # ---- chunk caches (fresh per layer)
                gat_cache = {}
                oh_cache = {}

                def get_gat(gt, lq=lq):
                    ci = int(tile2chunk[gt])
                    if ci not in gat_cache:
                        k, t0, ntile = chunks[ci]
                        buf = gp.tile([128, ntile * 128], bf16, tag="g")
                        nc.gpsimd.dma_gather(
                            out_ap=buf[:].rearrange("p (t e) -> p t e", e=128),
                            in_ap=h_full[lq][k][:],
                            idxs_ap=idx_t[:, t0 * 8:(t0 + ntile) * 8],
                            num_idxs=ntile * 128,
                            num_idxs_reg=ntile * 128,
                            elem_size=128,
                            single_packet=False,
                            queue_num=ci % NQ)
                        gat_cache[ci] = buf
                    k, t0, ntile = chunks[ci]
                    return gat_cache[ci][:].rearrange(
                        "p (t e) -> p t e", e=128)[:, gt - t0, :]

                def get_oh(gt):
                    ci = int(tile2chunk[gt])
                    if ci not in oh_cache:
                        k, t0, ntile = chunks[ci]
                        buf = ohp.tile([128, ntile * 128], bf16, tag="oh")
                        nc.sync.dma_start(
                            out=buf[:], in_=ohx_d[:, t0 * 128:(t0 + ntile) * 128])
                        oh_cache[ci] = buf
                    k, t0, ntile = chunks[ci]
                    return oh_cache[ci][:, (gt - t0) * 128:(gt - t0 + 1) * 128]

                # ---- phase A: bank-0 sweep -> bf16 partial aggregates
                for b in range(nb):
                    nt = int(T[b, 0])
                    ps = psa.tile([128, 128], fp32, tag="agg", space="PSUM")
                    for t in range(nt):
                        gt = int(off[0, b]) + t
                        nc.tensor.matmul(out=ps[:], lhsT=get_gat(gt),
                                         rhs=get_oh(gt),
                                         start=(t == 0), stop=(t == nt - 1))
                    nc.scalar.copy(out=aggb[b][:], in_=ps[:])

                # ---- phase B: bank-1 block-major, 3-stage pipelined finish
                aggF_d = {}
                s1_d = {}
                for i in range(nb + 2):
                    if i < nb:
                        b = i
                        nt = int(T[b, 1])
                        ps = psa.tile([128, 128], fp32, tag="agg", space="PSUM")
                        # re-inject bank-0 partial: I^T @ aggb = aggb
                        nc.tensor.matmul(out=ps[:], lhsT=ident_b[:],
                                         rhs=aggb[b][:], start=True, stop=False)
                        for t in range(nt):
                            gt = int(off[1, b]) + t
                            nc.tensor.matmul(out=ps[:], lhsT=get_gat(gt),
                                             rhs=get_oh(gt),
                                             start=False, stop=(t == nt - 1))
                        aggF = sbp.tile([128, 128], bf16, tag="aggF")
                        nc.scalar.copy(out=aggF[:], in_=ps[:])
                        aggF_d[b] = aggF
                    if 1 <= i <= nb:
                        b = i - 1
                        qaT = psp.tile([128, 128], fp32, tag="qa", space="PSUM")
                        nc.tensor.matmul(out=qaT[:], lhsT=qbig[:],
                                         rhs=aggF_d.pop(b)[:],
                                         start=True, stop=True)
                        qaTsb = sbp.tile([128, 128], fp32, tag="qaTsb")
                        nc.scalar.copy(out=qaTsb[:], in_=qaT[:])
                        bx = psp.tile([128, 128], fp32, tag="bx", space="PSUM")
                        nc.tensor.matmul(out=bx[:],
                                         lhsT=ohxt[:, b * 128:(b + 1) * 128],
                                         rhs=barrT[:], start=True, stop=True)
                        bxsb = sbp.tile([128, 128], fp32, tag="bxsb")
                        nc.scalar.copy(out=bxsb[:], in_=bx[:])
                        s1_d[b] = (qaTsb, bxsb)
                    if 2 <= i:
                        b = i - 2
                        qaTsb, bxsb = s1_d.pop(b)
                        qa2 = psp.tile([128, 128], fp32, tag="trp", space="PSUM")
                        nc.tensor.transpose(out=qa2[:], in_=qaTsb[:],
                                            identity=ident_f[:])
                        u = sbp.tile([128, 128], fp32, tag="u")
                        nc.vector.tensor_tensor(out=u[:], in0=qa2[:], in1=bxsb[:],
                                                op=OP.mult)
                        Z = sbp.tile([128, G], fp32, tag="Z")
                        nc.vector.reduce_sum(out=Z[:], in_=u[:].rearrange(
                            "p (g c) -> p g c", c=C), axis=AX)
                        nc.scalar.activation(
                            out=out_sb[:, b * 64 + l * G:b * 64 + (l + 1) * G],
                            in_=Z[:], func=AF.Ln)
                        if l < L - 1:
                            emit_h(l, b, u[:], Z)

            # ---- write lls out
            if nb > 1:
                nc.sync.dma_start(
                    out=lls_d[:(nb - 1) * 128, :].rearrange(
                        "(b p) c -> p b c", p=128),
                    in_=out_sb[:].rearrange("p (b c) -> p b c", c=64)[:, :nb - 1, :])
            nc.sync.dma_start(
                out=lls_d[(nb - 1) * 128:, :],
                in_=out_sb[:last_nn, (nb - 1) * 64:nb * 64])

    nc.compile()
    return nc


# ---- entry point ------------------------------------------------------------

def kernel(x, edge_index, lambda_B0, lambda_Pi, lambda_Q, lambda_B):
    cfg = Cfg()
    cores, sched, lncnt = preprocess(x, edge_index, cfg)
    consts = make_consts()
    nc = build_nc(cfg, sched)

    from concourse.bass_utils import run_bass_kernel_spmd
    params = permute_params(lambda_B0, lambda_Pi, lambda_Q, lambda_B)
    in_maps = []
    for c in range(cfg.ncores):
        m = dict(cores[c])
        m.update(params)
        m.update({k: np.ascontiguousarray(v) for k, v in consts.items()})
        in_maps.append(m)

    res = run_bass_kernel_spmd(nc, in_maps, core_ids=list(range(cfg.ncores)))
    out = np.concatenate([res.results[c]["lls"] for c in range(cfg.ncores)],
                         axis=0).reshape(N, L, G).astype(np.float32)
    out[:, 1:, :] -= lncnt[:, None, None]
    return out
